# revision 2
# baseline (speedup 1.0000x reference)
"""DGCNN forward on 8 Trainium2 NeuronCores via Bass/Tile (v2).

Sharding: data-parallel over graphs (B/8 = 128 graphs per core).

Host-side preparation is restricted to layout/index work (slicing per-core
shards, dense adjacency image encode, embedding gathers, integer degree
counts).  All model arithmetic runs on device.

v2 changes vs baseline:
  - GCN layers in A@(h@W) order: every matmul output is <=32 wide, so the
    fp32 PE cost per layer drops ~3x (no 256-wide fp32 matmul streams).
    W-multiplies and A-multiplies for a whole octo (8 graphs) accumulate
    into shared PSUM banks; ONE tanh per octo-layer with a strided output
    into the feat tile.
  - SortPooling selection: keys are transposed into [graph, node] layout
    (2 PE transposes for all 128 graphs), sorted with a cross-graph bitonic
    network (top-128 merge + descending merge), and the top-30 node indices
    are recovered by exact f32 equality match + iota/min reduction.
  - The 97-feature bitonic sort and the adjacency norm folds are split
    across the DVE and Pool(GPSIMD) engines.
"""

import sys

if "/opt/trn_rl_repo" not in sys.path:
    sys.path.insert(0, "/opt/trn_rl_repo")

import numpy as np

import concourse.bacc as bacc
import concourse.mybir as mybir
import concourse.tile as tile
from concourse.bass import IndirectOffsetOnAxis
from concourse.bass_utils import run_bass_kernel_spmd

F32 = mybir.dt.float32
BF16 = mybir.dt.bfloat16
I32 = mybir.dt.int32
AF = mybir.ActivationFunctionType
OP = mybir.AluOpType

N_ATTR = 100000
ATTR_DIM = 64
HID = 32
B = 1024
NPG = 256
EPG = 4096
TOPK = 30
DLAT = 97
N_CORES = 8
BIG = 3.0e38
BIGC = 100000.0  # index-recovery offset (exact in f32)

OCT_G = 8      # graphs per octo
N_OCT = 16     # octos per core (G=128)
S = 16         # 128-node slots per octo


# ---------------------------------------------------------------------------
# Device program
# ---------------------------------------------------------------------------

def build_program(G, num_devices, stage="full"):
    n_layers = 4
    assert G == 128
    nc = bacc.Bacc("TRN2", target_bir_lowering=False, debug=False,
                   num_devices=num_devices)

    def din(name, shape, dt=F32):
        return nc.dram_tensor(name, shape, dt, kind="ExternalInput").ap()

    x_imgT = din("x_imgT", [N_OCT, 128, S * 128])
    a_img = din("a_img", [G // 4, 128, 8 * NPG])
    dego = din("dego", [128, N_OCT * S])
    degi = din("degi", [128, N_OCT * S])
    ident = din("ident", [128, 128])
    iota0 = din("iota0", [128, 128])   # j - BIGC
    iota1 = din("iota1", [128, 128])   # 128 + j - BIGC
    gbase = din("gbase", [128, 1])     # g * NPG
    w0 = din("w0", [2 * HID + ATTR_DIM, HID])
    w1bd = din("w1bd", [128, 4 * HID])   # block-diag(W1 x4)
    w2bd = din("w2bd", [128, 4 * HID])
    w3bd = din("w3bd", [128, 4])
    c1wT = din("c1wT", [DLAT, 16], BF16)
    c1b = din("c1b", [16, 1])
    w2sb = din("w2sb", [16, 160])
    b2row = din("b2row", [1, 32])
    fc1wp = din("fc1wp", [128, 384])
    fc1b = din("fc1b", [128, 1])
    fc2wT = din("fc2wT", [128, 1])
    fc2b = din("fc2b", [1, 1])

    y = nc.dram_tensor("y", [G, 1], F32, kind="ExternalOutput").ap()
    feat_d = nc.dram_tensor("feat_d", [G * NPG, DLAT], F32).ap()
    # feat_d row (o*OCT_G*256 + q*256 + c*128 + p) view for per-octo writes
    fdv = feat_d.rearrange("(o q c p) d -> o c p q d", q=OCT_G, c=2, p=128)

    LAYER_DOUT = (HID, HID, HID, 1)
    FBASE = (0, 512, 1024, 1536)

    with tile.TileContext(nc) as tc:
        with tc.tile_pool(name="cst", bufs=1) as cst:
            def load_const(src, shape):
                t = cst.tile(shape, src.dtype, tag=f"c{src.tensor.name}")
                nc.sync.dma_start(out=t[:], in_=src)
                return t

            identity = load_const(ident[:], [128, 128])
            identb = cst.tile([128, 128], BF16, tag="identb")
            nc.vector.tensor_scalar(out=identb[:], in0=identity[:],
                                    scalar1=0.0, scalar2=None, op0=OP.add)
            iota0_s = load_const(iota0[:], [128, 128])
            iota1_s = load_const(iota1[:], [128, 128])
            gb_s = load_const(gbase[:], [128, 1])
            ones_row = cst.tile([1, 128], F32, tag="ones_row")
            nc.vector.memset(ones_row[:], 1.0)
            permI = cst.tile([128, 32], I32, tag="permI")
            keys_all = cst.tile([128, N_OCT * S], F32, tag="keys_all")

            w0_s = load_const(w0[:], [128, HID])
            w1bd_s = load_const(w1bd[:], [128, 4 * HID])
            w2bd_s = load_const(w2bd[:], [128, 4 * HID])
            w3bd_s = load_const(w3bd[:], [128, 4])
            c1w_s = load_const(c1wT[:], [DLAT, 16])
            c1b_s = load_const(c1b[:], [16, 1])
            w2sb_s = load_const(w2sb[:], [16, 160])
            b2r_s = load_const(b2row[:], [1, 32])
            fc1w_s = load_const(fc1wp[:], [128, 384])
            fc1b_s = load_const(fc1b[:], [128, 1])
            fc2w_s = load_const(fc2wT[:], [128, 1])
            fc2b_s = load_const(fc2b[:], [1, 1])
            LWBD = (None, w1bd_s, w2bd_s, w3bd_s)

            # degree norms for ALL octos upfront
            ns_all = cst.tile([128, N_OCT * S], F32, tag="ns_all")
            nd_all = cst.tile([128, N_OCT * S], F32, tag="nd_all")
            for (dsrc, dt_) in ((dego, ns_all), (degi, nd_all)):
                nc.sync.dma_start(out=dt_[:], in_=dsrc)
                nc.vector.tensor_scalar_max(dt_[:], dt_[:], 1.0)
                nc.vector.reciprocal(dt_[:], dt_[:])
                nc.scalar.activation(dt_[:], dt_[:], AF.Sqrt)

            # ------------ Stage B: GCN layers (A@(h@W) order) --------------
            # norms fold into the data path: m' = ns (.) (h@W) on the PSUM
            # exit of the W-multiply, u' = nd (.) (A@m') before tanh --
            # A stays RAW (no per-element adjacency scaling needed).
            with tc.tile_pool(name="octo", bufs=4) as opool, \
                 tc.tile_pool(name="adj", bufs=8) as rpool, \
                 tc.tile_pool(name="gwork", bufs=10) as gpool, \
                 tc.tile_pool(name="psMm", bufs=3, space="PSUM") as ppMm, \
                 tc.tile_pool(name="psMa", bufs=3, space="PSUM") as ppMa, \
                 tc.tile_pool(name="psH", bufs=2, space="PSUM") as ppH:
                hts_rot = [0]

                def octo_prologue(o):
                    st = {}
                    st["o"] = o
                    st["ns8"] = ns_all[:, o * S:(o + 1) * S]
                    st["nd8"] = nd_all[:, o * S:(o + 1) * S]
                    xT8 = opool.tile([128, S * 128], F32, tag="xT8",
                                     name=f"xT8_{o}")
                    st["xT8"] = xT8
                    # layer-major feature blocks: col = FBASE[l]+s*dout+d
                    st["feat"] = opool.tile([128, S * DLAT], F32, tag="feat",
                                            name=f"feat_{o}")
                    nc.sync.dma_start(out=xT8[:], in_=x_imgT[o])
                    at = []
                    for half in range(2):
                        g0 = o * OCT_G + half * 4
                        at4 = rpool.tile([128, 8 * NPG], F32, tag="araw",
                                         name=f"araw_{o}_{half}")
                        nc.scalar.dma_start(out=at4[:], in_=a_img[g0 // 4])
                        at.append(at4)
                    st["at"] = at
                    return st

                def layer_m(st, li):
                    o, feat, xT8 = st["o"], st["feat"], st["xT8"]
                    dout = LAYER_DOUT[li]
                    bank_m = ppMm.tile([128, 16 * dout], F32, tag="bkm",
                                       name=f"bkm_{o}_{li}")
                    st["bank_m"] = bank_m
                    if li == 0:
                        for s in range(S):
                            nc.tensor.matmul(
                                out=bank_m[:, s * dout:(s + 1) * dout],
                                lhsT=xT8[:, s * 128:(s + 1) * 128],
                                rhs=w0_s[:, :dout],
                                start=True, stop=True)
                    else:
                        pb = FBASE[li - 1]
                        wbd = LWBD[li]
                        for grp in range(4):
                            htp = ppH.tile([128, 128], F32, tag="htp",
                                           name=f"htp_{o}_{li}_{grp}")
                            nc.tensor.transpose(
                                out=htp[:],
                                in_=feat[:, pb + grp * 128:
                                         pb + grp * 128 + 128],
                                identity=identity[:])
                            hts = gpool.tile([128, 128], F32, tag="hts",
                                             name=f"hts_{o}_{li}_{grp}")
                            nc.scalar.copy(out=hts[:], in_=htp[:])
                            nc.tensor.matmul(
                                out=bank_m[:, grp * 4 * dout:
                                           (grp + 1) * 4 * dout],
                                lhsT=hts[:],
                                rhs=wbd[:, :4 * dout],
                                start=True, stop=True)

                def layer_ns(st, li):
                    o = st["o"]
                    dout = LAYER_DOUT[li]
                    m_sb = gpool.tile([128, 16 * dout], F32, tag="msb",
                                      name=f"msb_{o}_{li}")
                    st["m_sb"] = m_sb
                    nc.vector.tensor_tensor(
                        out=m_sb[:].rearrange("p (s d) -> p s d", d=dout),
                        in0=st["bank_m"][:]
                        .rearrange("p (s d) -> p s d", d=dout),
                        in1=st["ns8"]
                        .rearrange("p (s one) -> p s one", one=1)
                        .to_broadcast([128, S, dout]),
                        op=OP.mult)

                def layer_a(st, li):
                    o = st["o"]
                    dout = LAYER_DOUT[li]
                    m_sb = st["m_sb"]

                    def a_slice(q, sc, dc):
                        t = st["at"][q // 4]
                        qq = q % 4
                        base = (2 * qq + sc) * NPG + dc * 128
                        return t[:, base:base + 128]

                    bank_a = ppMa.tile([128, 16 * dout], F32, tag="bka",
                                       name=f"bka_{o}_{li}")
                    st["bank_a"] = bank_a
                    for q in range(OCT_G):
                        for dc in range(2):
                            od = (2 * q + dc) * dout
                            for sc in range(2):
                                nc.tensor.matmul(
                                    out=bank_a[:, od:od + dout],
                                    lhsT=a_slice(q, sc, dc),
                                    rhs=m_sb[:, (2 * q + sc) * dout:
                                             (2 * q + sc + 1) * dout],
                                    start=(sc == 0), stop=(sc == 1))

                def layer_nd(st, li):
                    o = st["o"]
                    dout = LAYER_DOUT[li]
                    u_sb = gpool.tile([128, 16 * dout], F32, tag="usb",
                                      name=f"usb_{o}_{li}")
                    st["u_sb"] = u_sb
                    nc.vector.tensor_tensor(
                        out=u_sb[:].rearrange("p (s d) -> p s d", d=dout),
                        in0=st["bank_a"][:]
                        .rearrange("p (s d) -> p s d", d=dout),
                        in1=st["nd8"]
                        .rearrange("p (s one) -> p s one", one=1)
                        .to_broadcast([128, S, dout]),
                        op=OP.mult)

                def layer_tanh(st, li):
                    dout = LAYER_DOUT[li]
                    fb = FBASE[li]
                    nc.scalar.activation(
                        out=st["feat"][:, fb:fb + S * dout],
                        in_=st["u_sb"][:],
                        func=AF.Tanh)

                def octo_epilogue(st):
                    o, feat = st["o"], st["feat"]
                    if n_layers < 4:
                        nc.vector.tensor_scalar(
                            out=keys_all[:, o * S:(o + 1) * S],
                            in0=feat[:, 0:S], scalar1=0.0, scalar2=None,
                            op0=OP.add)
                        return
                    # keys = max over the 97 features
                    kb = gpool.tile([128, 3 * S], F32, tag="kb",
                                    name=f"kb_{o}")
                    for l in range(3):
                        nc.vector.tensor_reduce(
                            out=kb[:, l * S:(l + 1) * S]
                            .rearrange("p (s one) -> p s one", one=1),
                            in_=feat[:, l * 512:(l + 1) * 512]
                            .rearrange("p (s d) -> p s d", d=32),
                            axis=mybir.AxisListType.X, op=OP.max)
                    nc.vector.tensor_tensor(out=kb[:, 0:S], in0=kb[:, 0:S],
                                            in1=kb[:, S:2 * S], op=OP.max)
                    nc.vector.tensor_tensor(out=kb[:, 0:S], in0=kb[:, 0:S],
                                            in1=kb[:, 2 * S:3 * S],
                                            op=OP.max)
                    nc.vector.tensor_tensor(
                        out=keys_all[:, o * S:(o + 1) * S],
                        in0=kb[:, 0:S], in1=feat[:, 1536:1552], op=OP.max)

                    # write feat rows (columns in layer-major permuted
                    # order -- harmless, the 97-sort reorders them)
                    for c in range(2):
                        for l in range(3):
                            nc.sync.dma_start(
                                out=fdv[o, c][:, :, l * 32:(l + 1) * 32],
                                in_=feat[:, l * 512:(l + 1) * 512]
                                .rearrange("p (s d) -> p s d", d=32)
                                [:, c::2, :])
                        nc.sync.dma_start(
                            out=fdv[o, c][:, :, 96:97],
                            in_=feat[:, 1536:1552][:, c::2]
                            .rearrange("p (q one) -> p q one", one=1))

                # emit octo PAIRS with layer-interleaved instruction order:
                # per-engine streams are in-order, so alternating the two
                # independent octos' stages lets each engine fill the other
                # octo's cross-engine dependency gaps.
                GRPN = 2
                o_iter = iter(range(N_OCT))
                groups = []
                rem = N_OCT
                while rem > 0:
                    g = min(GRPN, rem)
                    groups.append([next(o_iter) for _ in range(g)])
                    rem -= g
                for grp_os in groups:
                    sts = [octo_prologue(o) for o in grp_os]
                    for li in range(n_layers):
                        for fn in (layer_m, layer_ns, layer_a, layer_nd,
                                   layer_tanh):
                            for st in sts:
                                fn(st, li)
                    for st in sts:
                        octo_epilogue(st)

            run_C = stage in ("C", "C2", "full")
            run_C2 = stage in ("C2", "full")
            run_D = stage == "full"
            if stage == "B":
                nc.sync.dma_start(out=y[:, 0:1], in_=keys_all[0:1, 0:G])

            # ------------ Stage C: top-30 selection ------------------------
            if run_C:
              with tc.tile_pool(name="selp", bufs=1) as sp, \
                 tc.tile_pool(name="selps", bufs=2, space="PSUM") as spp:
                kG = []
                srtb = []
                for h in range(2):
                    tp = spp.tile([128, 128], F32, tag="ktp")
                    nc.tensor.transpose(out=tp[:],
                                        in_=keys_all[:, h::2],
                                        identity=identity[:])
                    kg = sp.tile([128, 128], F32, tag=f"kg{h}")
                    nc.scalar.copy(out=kg[:], in_=tp[:])
                    sb = [sp.tile([128, 128], F32, tag=f"s{h}{i}",
                                  name=f"sort{h}_{i}")
                          for i in range(2)]
                    nc.vector.tensor_scalar(out=sb[0][:], in0=kg[:],
                                            scalar1=0.0, scalar2=None,
                                            op0=OP.add)
                    kG.append(kg)
                    srtb.append(sb)

                def ce_steps(n):
                    steps = []
                    klog = 1
                    while (1 << klog) <= n:
                        bs = 1 << klog
                        steps.append(("flip", bs))
                        for jj in range(klog - 2, -1, -1):
                            steps.append(("plain", 1 << jj))
                        klog += 1
                    return steps

                def bitonic_pass(sb, steps, asc, engines):
                    cur = 0
                    for (kind, d) in steps:
                        a, b_ = sb[cur][:], sb[1 - cur][:]
                        if kind == "flip":
                            bs = d
                            half = bs // 2
                            ai = a.rearrange("p (b x) -> p b x", x=bs)
                            bi = b_.rearrange("p (b x) -> p b x", x=bs)
                            lo_in = ai[:, :, 0:half]
                            hi_in = ai[:, :, bs - 1:half - 1:-1]
                            lo_out = bi[:, :, 0:half]
                            hi_out = bi[:, :, bs - 1:half - 1:-1]
                        else:
                            blk = 2 * d
                            ai = a.rearrange("p (b x) -> p b x", x=blk)
                            bi = b_.rearrange("p (b x) -> p b x", x=blk)
                            lo_in = ai[:, :, 0:d]
                            hi_in = ai[:, :, d:blk]
                            lo_out = bi[:, :, 0:d]
                            hi_out = bi[:, :, d:blk]
                        lo_op, hi_op = (OP.min, OP.max) if asc \
                            else (OP.max, OP.min)
                        engines[0].tensor_tensor(out=lo_out, in0=lo_in,
                                                 in1=hi_in, op=lo_op)
                        engines[1].tensor_tensor(out=hi_out, in0=lo_in,
                                                 in1=hi_in, op=hi_op)
                        cur = 1 - cur
                    return cur

                steps128 = ce_steps(128)
                c0 = bitonic_pass(srtb[0], steps128, True,
                                  (nc.vector, nc.vector))
                c1 = bitonic_pass(srtb[1], steps128, False,
                                  (nc.vector, nc.vector))
                # merge: hi half = top-128 (bitonic)
                hi = [sp.tile([128, 128], F32, tag=f"hi{i}",
                              name=f"hi_{i}")
                      for i in range(2)]
                nc.vector.tensor_tensor(out=hi[0][:], in0=srtb[0][c0][:],
                                        in1=srtb[1][c1][:], op=OP.max)
                # descending bitonic merge of hi
                mcur = 0
                for d in (64, 32, 16, 8, 4, 2, 1):
                    blk = 2 * d
                    a, b_ = hi[mcur][:], hi[1 - mcur][:]
                    ai = a.rearrange("p (b x) -> p b x", x=blk)
                    bi = b_.rearrange("p (b x) -> p b x", x=blk)
                    nc.vector.tensor_tensor(out=bi[:, :, 0:d],
                                            in0=ai[:, :, 0:d],
                                            in1=ai[:, :, d:blk], op=OP.max)
                    nc.vector.tensor_tensor(out=bi[:, :, d:blk],
                                            in0=ai[:, :, 0:d],
                                            in1=ai[:, :, d:blk], op=OP.min)
                    mcur = 1 - mcur
                top = hi[mcur]  # [:, 0:30] descending

                # index recovery: sel = (kG == top_r) * (iota - BIGC);
                # min over nodes -> idx - BIGC
                selw = sp.tile([128, TOPK * 256], F32, tag="selw")
                for r in range(TOPK):
                    for h in range(2):
                        eng = nc.vector
                        eng.scalar_tensor_tensor(
                            out=selw[:, r * 256 + h * 128:
                                     r * 256 + (h + 1) * 128],
                            in0=kG[h][:], scalar=top[:, r:r + 1],
                            in1=(iota0_s if h == 0 else iota1_s)[:],
                            op0=OP.is_equal, op1=OP.mult)
                idxm = sp.tile([128, TOPK], F32, tag="idxm")
                nc.vector.tensor_reduce(
                    out=idxm[:].rearrange("p (r one) -> p r one", one=1),
                    in_=selw[:].rearrange("p (r n) -> p r n", n=256),
                    axis=mybir.AxisListType.X, op=OP.min)
                nc.vector.tensor_scalar(out=idxm[:], in0=idxm[:],
                                        scalar1=BIGC, scalar2=None,
                                        op0=OP.add)
                nc.vector.tensor_scalar(out=idxm[:], in0=idxm[:],
                                        scalar1=gb_s[:], scalar2=None,
                                        op0=OP.add)
                # clamp to the valid feat_d row range (turns any upstream
                # numeric surprise into a wrong row instead of an OOB DMA)
                nc.vector.tensor_scalar_max(idxm[:], idxm[:], 0.0)
                nc.vector.tensor_scalar_min(idxm[:], idxm[:],
                                            float(G * NPG - 1))
                nc.vector.tensor_scalar(out=permI[:, :TOPK], in0=idxm[:],
                                        scalar1=0.0, scalar2=None,
                                        op0=OP.add)

            if stage == "C":
                with tc.tile_pool(name="dummy", bufs=1) as dp:
                    ysC = dp.tile([128, 1], F32, tag="ysC")
                    nc.vector.tensor_scalar(out=ysC[:], in0=permI[:, 0:1],
                                            scalar1=0.0, scalar2=None,
                                            op0=OP.add)
                    nc.sync.dma_start(out=y[:, 0:1], in_=ysC[:, 0:1])

            # ------------ Stage C2: fetch top-30 rows + 97-sort ------------
            if run_C2:
              with tc.tile_pool(name="sortp", bufs=1) as spool:
                srtf = spool.tile([G, TOPK * 128], F32, tag="srtf")
                svf = srtf[:].rearrange("p (c n) -> p c n", n=128)
                for r in range(TOPK):
                    nc.gpsimd.indirect_dma_start(
                        out=svf[:, r, 0:DLAT], out_offset=None,
                        in_=feat_d[:],
                        in_offset=IndirectOffsetOnAxis(
                            ap=permI[:G, r:r + 1], axis=0))
                # bf16 copies for the feature sort (order-only precision)
                srt = [spool.tile([G, TOPK * 128], BF16, tag=f"s{i}",
                                  name=f"srt{i}")
                       for i in range(2)]
                nc.vector.memset(srt[0][:], BIG)
                sv = [t[:].rearrange("p (c n) -> p c n", n=128) for t in srt]
                nc.vector.tensor_scalar(
                    out=sv[0][:, :, 0:DLAT], in0=svf[:, :, 0:DLAT],
                    scalar1=0.0, scalar2=None, op0=OP.add)

                CSPLIT = 17
                cur = 0
                for (kind, d) in ce_steps(128):
                    a, b_ = sv[cur], sv[1 - cur]
                    if kind == "flip":
                        bs = d
                        half = bs // 2
                        ai = a.rearrange("p c (b x) -> p c b x", x=bs)
                        bi = b_.rearrange("p c (b x) -> p c b x", x=bs)
                        lo_in = ai[:, :, :, 0:half]
                        hi_in = ai[:, :, :, bs - 1:half - 1:-1]
                        lo_out = bi[:, :, :, 0:half]
                        hi_out = bi[:, :, :, bs - 1:half - 1:-1]
                    else:
                        blk = 2 * d
                        ai = a.rearrange("p c (b x) -> p c b x", x=blk)
                        bi = b_.rearrange("p c (b x) -> p c b x", x=blk)
                        lo_in = ai[:, :, :, 0:d]
                        hi_in = ai[:, :, :, d:blk]
                        lo_out = bi[:, :, :, 0:d]
                        hi_out = bi[:, :, :, d:blk]
                    nc.vector.tensor_tensor(out=lo_out, in0=lo_in,
                                            in1=hi_in, op=OP.min)
                    nc.vector.tensor_tensor(out=hi_out, in0=lo_in,
                                            in1=hi_in, op=OP.max)
                    cur = 1 - cur

                if stage == "C2":
                    ysD = spool.tile([128, 1], F32, tag="ysD")
                    nc.vector.tensor_reduce(
                        out=ysD[:].rearrange("p (a one) -> p a one", one=1),
                        in_=sv[cur][:, 0:1, 0:DLAT],
                        axis=mybir.AxisListType.X, op=OP.max)
                    nc.sync.dma_start(out=y[:, 0:1], in_=ysD[:, 0:1])

                # ------------ Stage D: CNN + MLP ---------------------------
                if run_D:
                  with tc.tile_pool(name="cnn", bufs=2) as cp, \
                     tc.tile_pool(name="cnnp", bufs=2, space="PSUM") as cpp:
                    z1T = spool.tile([16, TOPK * G], F32, tag="z1T")
                    sfin = sv[cur]
                    for ch in range(TOPK):
                        tp = cpp.tile([128, G], BF16, tag="ctp")
                        nc.tensor.transpose(out=tp[:, :G],
                                            in_=sfin[:, ch, :],
                                            identity=identb[:G, :G])
                        ps = cp.tile([DLAT, G], BF16, tag="ps")
                        nc.scalar.copy(out=ps[:], in_=tp[:DLAT, :G])
                        zp = cpp.tile([16, G], F32, tag="zsm")
                        nc.tensor.matmul(out=zp[:], lhsT=c1w_s[:], rhs=ps[:],
                                         start=True, stop=True)
                        nc.scalar.activation(z1T[:, ch * G:(ch + 1) * G],
                                             zp[:], AF.Relu, bias=c1b_s[:])

                    z2T = spool.tile([16, 15 * G], F32, tag="z2T")
                    z1v = z1T[:].rearrange("p (c g) -> p c g", g=G)
                    nc.vector.tensor_tensor(
                        out=z2T[:].rearrange("p (c g) -> p c g", g=G),
                        in0=z1v[:, 0:30:2, :], in1=z1v[:, 1:30:2, :],
                        op=OP.max)

                    zperm = spool.tile([G, 352], F32, tag="zperm")
                    for j in range(11):
                        z3 = cpp.tile([G, 32], F32, tag="zsm")
                        for t in range(5):
                            nc.tensor.matmul(
                                out=z3[:],
                                lhsT=z2T[:, (j + t) * G:(j + t + 1) * G],
                                rhs=w2sb_s[:, 32 * t:32 * t + 32],
                                start=(t == 0), stop=False)
                        nc.tensor.matmul(out=z3[:], lhsT=ones_row[:1, :G],
                                         rhs=b2r_s[:], start=False, stop=True)
                        nc.scalar.activation(zperm[:, 32 * j:32 * j + 32],
                                             z3[:], AF.Relu)

                    zts = []
                    for c in range(3):
                        w = min(128, 352 - 128 * c)
                        tp = cpp.tile([128, G], F32, tag="ctp")
                        nc.tensor.transpose(out=tp[:w, :G],
                                            in_=zperm[:, 128 * c:128 * c + w],
                                            identity=identity[:G, :G])
                        zt = cp.tile([128, G], F32, tag=f"zt{c}")
                        nc.scalar.copy(out=zt[:w, :], in_=tp[:w, :G])
                        zts.append((zt, w))
                    upf = cpp.tile([128, G], F32, tag="fc1")
                    for c, (zt, w) in enumerate(zts):
                        nc.tensor.matmul(
                            out=upf[:],
                            lhsT=fc1w_s[:w, 128 * c:128 * c + 128],
                            rhs=zt[:w, :], start=(c == 0), stop=(c == 2))
                    us = cp.tile([128, G], F32, tag="us")
                    nc.scalar.activation(us[:], upf[:], AF.Relu,
                                         bias=fc1b_s[:])
                    ypp = cpp.tile([1, G], F32, tag="zsm")
                    nc.tensor.matmul(out=ypp[:], lhsT=fc2w_s[:], rhs=us[:],
                                     start=True, stop=True)
                    ys = cp.tile([1, G], F32, tag="ys")
                    nc.scalar.activation(ys[:], ypp[:], AF.Identity,
                                         bias=fc2b_s[:])
                    nc.sync.dma_start(out=y[:, 0:1], in_=ys[0:1, :])

    nc.compile()
    return nc


# ---------------------------------------------------------------------------
# Host-side layout preparation + sharding
# ---------------------------------------------------------------------------

def _prep_core(c, G, x_full, ew, src, dst, deg_o, deg_i):
    g0 = c * G
    nsl = slice(g0 * NPG, (g0 + G) * NPG)
    esl = slice(g0 * EPG, (g0 + G) * EPG)

    el = np.arange(G * EPG, dtype=np.int64)
    gl = el // EPG
    src_l = np.asarray(src[esl], np.int64) - g0 * NPG - gl * NPG
    dst_l = np.asarray(dst[esl], np.int64) - g0 * NPG - gl * NPG
    assert src_l.min() >= 0 and src_l.max() < NPG
    assert dst_l.min() >= 0 and dst_l.max() < NPG

    # dense adjacency image, [g*256 + src, dst], then retile to
    # [tile=4 graphs][128 p, (k=8 chunks, 256 d)] for contiguous DMA loads
    cell = (gl * NPG + src_l) * NPG + dst_l
    a_img = np.bincount(cell, weights=ew[esl].astype(np.float64),
                        minlength=G * NPG * NPG)
    a_img = a_img.reshape(G * NPG, NPG).astype(np.float32)
    a_img = np.ascontiguousarray(
        a_img.reshape(G // 4, 8, 128, NPG).transpose(0, 2, 1, 3)
        .reshape(G // 4, 128, 8 * NPG))

    # slot layout: node(o, s, p) = (o*OCT_G + s//2)*NPG + (s%2)*128 + p
    p = np.arange(128)[:, None]
    sidx = np.arange(2 * OCT_G)[None, :]
    o = np.arange(N_OCT)[:, None, None]
    node = (o * OCT_G + sidx // 2) * NPG + (sidx % 2) * 128 + p
    xc = x_full[nsl]
    # x_imgT[o][f, s*128 + p] = x[node(o,s,p), f]
    xg = xc[node]                          # [n_oct, 128(p), S, 128(f)]
    x_imgT = np.ascontiguousarray(
        xg.transpose(0, 3, 2, 1).reshape(N_OCT, 128, S * 128))

    def deg_layout(d):
        a = d[nsl][node].astype(np.float32)
        return np.ascontiguousarray(a.transpose(1, 0, 2).reshape(128, -1))

    return dict(
        x_imgT=x_imgT.astype(np.float32), a_img=a_img,
        dego=deg_layout(deg_o), degi=deg_layout(deg_i))


def _bd4(W):
    W = np.asarray(W, np.float32)
    k, d = W.shape
    out = np.zeros((4 * k, 4 * d), np.float32)
    for i in range(4):
        out[i * k:(i + 1) * k, i * d:(i + 1) * d] = W
    return np.ascontiguousarray(out)


def _prep_weights(inp):
    f32 = lambda a: np.ascontiguousarray(np.asarray(a), np.float32)
    conv1_w = np.asarray(inp["conv1_w"], np.float32)
    conv2_w = np.asarray(inp["conv2_w"], np.float32)
    fc1_w = np.asarray(inp["fc1_w"], np.float32)

    import ml_dtypes
    c1wT = np.ascontiguousarray(conv1_w[:, 0, :].T.astype(ml_dtypes.bfloat16))
    w2sb = f32(np.transpose(conv2_w, (1, 2, 0)).reshape(16, 160))
    perm = np.empty(352, np.int64)
    for c2 in range(32):
        for j in range(11):
            perm[j * 32 + c2] = c2 * 11 + j
    fc1c = fc1_w[:, perm].T  # [352, 128] K-major
    packed = np.zeros((128, 384), np.float32)
    for c in range(3):
        w = min(128, 352 - 128 * c)
        packed[:w, 128 * c:128 * c + 128] = fc1c[128 * c:128 * c + w, :]
    jj = np.arange(128, dtype=np.float32)[None, :]
    return dict(
        ident=np.eye(128, dtype=np.float32),
        iota0=np.ascontiguousarray(np.tile(jj - BIGC, (128, 1))),
        iota1=np.ascontiguousarray(np.tile(jj + 128.0 - BIGC, (128, 1))),
        gbase=(np.arange(128, dtype=np.float32) * NPG)[:, None],
        w0=f32(inp["W0"]),
        w1bd=_bd4(inp["W1"]), w2bd=_bd4(inp["W2"]), w3bd=_bd4(inp["W3"]),
        c1wT=c1wT, c1b=f32(inp["conv1_b"]).reshape(-1, 1), w2sb=w2sb,
        b2row=f32(inp["conv2_b"]).reshape(1, -1), fc1wp=packed,
        fc1b=f32(inp["fc1_b"]).reshape(-1, 1), fc2wT=f32(inp["fc2_w"].T),
        fc2b=f32(inp["fc2_b"]).reshape(-1, 1))


def make_in_maps(inputs, G, n_cores):
    feats = np.asarray(inputs["feats"], np.int64)
    node_id = np.asarray(inputs["node_id"], np.int64)
    edge_id = np.asarray(inputs["edge_id"], np.int64)
    src = np.asarray(inputs["src"], np.int64)
    dst = np.asarray(inputs["dst"], np.int64)
    ndata = np.asarray(inputs["ndata"], np.float32)
    node_emb = np.asarray(inputs["node_emb"], np.float32)
    edata = np.asarray(inputs["edata"], np.float32)
    N = feats.shape[0]

    deg_o = np.bincount(src, minlength=N).astype(np.float32)
    deg_i = np.bincount(dst, minlength=N).astype(np.float32)
    x_full = np.concatenate(
        [ndata[node_id], node_emb[feats], node_emb[N_ATTR + node_id]],
        axis=1).astype(np.float32)
    ew = edata[edge_id][:, 0]

    shared = _prep_weights(inputs)
    in_maps = []
    for c in range(n_cores):
        m = dict(shared)
        m.update(_prep_core(c, G, x_full, ew, src, dst, deg_o, deg_i))
        in_maps.append(m)
    return in_maps


_PROG_CACHE = {}


def _get_program(G, num_devices, stage="full"):
    key = (G, num_devices, stage)
    if key not in _PROG_CACHE:
        _PROG_CACHE[key] = build_program(G, num_devices, stage)
    return _PROG_CACHE[key]


def kernel(**inputs):
    G = B // N_CORES
    nc = _get_program(G, N_CORES)
    in_maps = make_in_maps(inputs, G, N_CORES)
    res = run_bass_kernel_spmd(nc, in_maps, list(range(N_CORES)))
    out = np.concatenate([res.results[c]["y"] for c in range(N_CORES)],
                         axis=0)
    return out.astype(np.float32)


# revision 3
# speedup vs baseline: 2.0512x; 2.0512x over previous
"""DGCNN forward on 8 Trainium2 NeuronCores via Bass/Tile (v2).

Sharding: data-parallel over graphs (B/8 = 128 graphs per core).

Host-side preparation is restricted to layout/index work (slicing per-core
shards, dense adjacency image encode, embedding gathers, integer degree
counts).  All model arithmetic runs on device.

v2 changes vs baseline:
  - GCN layers in A@(h@W) order: every matmul output is <=32 wide, so the
    fp32 PE cost per layer drops ~3x (no 256-wide fp32 matmul streams).
    W-multiplies and A-multiplies for a whole octo (8 graphs) accumulate
    into shared PSUM banks; ONE tanh per octo-layer with a strided output
    into the feat tile.
  - SortPooling selection: keys are transposed into [graph, node] layout
    (2 PE transposes for all 128 graphs), sorted with a cross-graph bitonic
    network (top-128 merge + descending merge), and the top-30 node indices
    are recovered by exact f32 equality match + iota/min reduction.
  - The 97-feature bitonic sort and the adjacency norm folds are split
    across the DVE and Pool(GPSIMD) engines.
"""

import sys

if "/opt/trn_rl_repo" not in sys.path:
    sys.path.insert(0, "/opt/trn_rl_repo")

import numpy as np

import concourse.bacc as bacc
import concourse.mybir as mybir
import concourse.tile as tile
from concourse.bass import IndirectOffsetOnAxis
from concourse.bass_utils import run_bass_kernel_spmd

F32 = mybir.dt.float32
BF16 = mybir.dt.bfloat16
I32 = mybir.dt.int32
AF = mybir.ActivationFunctionType
OP = mybir.AluOpType

N_ATTR = 100000
ATTR_DIM = 64
HID = 32
B = 1024
NPG = 256
EPG = 4096
TOPK = 30
DLAT = 97
N_CORES = 8
BIG = 3.0e38
BIGC = 100000.0  # index-recovery offset (exact in f32)

OCT_G = 8      # graphs per octo
N_OCT = 16     # octos per core (G=128)
S = 16         # 128-node slots per octo


# ---------------------------------------------------------------------------
# Device program
# ---------------------------------------------------------------------------

def build_program(G, num_devices, stage="full"):
    n_layers = 4
    assert G == 128
    nc = bacc.Bacc("TRN2", target_bir_lowering=False, debug=False,
                   num_devices=num_devices)

    def din(name, shape, dt=F32):
        return nc.dram_tensor(name, shape, dt, kind="ExternalInput").ap()

    x_imgT = din("x_imgT", [N_OCT, 128, S * 128])
    a_img = din("a_img", [G // 4, 128, 8 * NPG])
    dego = din("dego", [128, N_OCT * S])
    degi = din("degi", [128, N_OCT * S])
    ident = din("ident", [128, 128])
    iota0 = din("iota0", [128, 128])   # j - BIGC
    iota1 = din("iota1", [128, 128])   # 128 + j - BIGC
    gbase = din("gbase", [128, 1])     # g * NPG
    w0 = din("w0", [2 * HID + ATTR_DIM, HID])
    w1bd = din("w1bd", [128, 4 * HID])   # block-diag(W1 x4)
    w2bd = din("w2bd", [128, 4 * HID])
    w3bd = din("w3bd", [128, 4])
    c1wT = din("c1wT", [DLAT, 16], BF16)
    c1b = din("c1b", [16, 1])
    w2sb = din("w2sb", [16, 160])
    b2row = din("b2row", [1, 32])
    fc1wp = din("fc1wp", [128, 384])
    fc1b = din("fc1b", [128, 1])
    fc2wT = din("fc2wT", [128, 1])
    fc2b = din("fc2b", [1, 1])

    y = nc.dram_tensor("y", [G, 1], F32, kind="ExternalOutput").ap()
    feat_d = nc.dram_tensor("feat_d", [G * NPG, DLAT], F32).ap()
    # feat_d row (o*OCT_G*256 + q*256 + c*128 + p) view for per-octo writes
    fdv = feat_d.rearrange("(o q c p) d -> o c p q d", q=OCT_G, c=2, p=128)

    LAYER_DOUT = (HID, HID, HID, 1)
    FBASE = (0, 512, 1024, 1536)

    with tile.TileContext(nc) as tc:
        with tc.tile_pool(name="cst", bufs=1) as cst:
            def load_const(src, shape):
                t = cst.tile(shape, src.dtype, tag=f"c{src.tensor.name}")
                nc.sync.dma_start(out=t[:], in_=src)
                return t

            identity = load_const(ident[:], [128, 128])
            identb = cst.tile([128, 128], BF16, tag="identb")
            nc.vector.tensor_scalar(out=identb[:], in0=identity[:],
                                    scalar1=0.0, scalar2=None, op0=OP.add)
            iota0_s = load_const(iota0[:], [128, 128])
            iota1_s = load_const(iota1[:], [128, 128])
            gb_s = load_const(gbase[:], [128, 1])
            ones_row = cst.tile([1, 128], F32, tag="ones_row")
            nc.vector.memset(ones_row[:], 1.0)
            permI = cst.tile([128, 32], I32, tag="permI")
            keys_all = cst.tile([128, N_OCT * S], F32, tag="keys_all")

            w0_s = load_const(w0[:], [128, HID])
            w1bd_s = load_const(w1bd[:], [128, 4 * HID])
            w2bd_s = load_const(w2bd[:], [128, 4 * HID])
            w3bd_s = load_const(w3bd[:], [128, 4])
            c1w_s = load_const(c1wT[:], [DLAT, 16])
            c1b_s = load_const(c1b[:], [16, 1])
            w2sb_s = load_const(w2sb[:], [16, 160])
            b2r_s = load_const(b2row[:], [1, 32])
            fc1w_s = load_const(fc1wp[:], [128, 384])
            fc1b_s = load_const(fc1b[:], [128, 1])
            fc2w_s = load_const(fc2wT[:], [128, 1])
            fc2b_s = load_const(fc2b[:], [1, 1])
            LWBD = (None, w1bd_s, w2bd_s, w3bd_s)

            # degree norms for ALL octos upfront
            ns_all = cst.tile([128, N_OCT * S], F32, tag="ns_all")
            nd_all = cst.tile([128, N_OCT * S], F32, tag="nd_all")
            for (dsrc, dt_) in ((dego, ns_all), (degi, nd_all)):
                nc.sync.dma_start(out=dt_[:], in_=dsrc)
                nc.vector.tensor_scalar_max(dt_[:], dt_[:], 1.0)
                nc.vector.reciprocal(dt_[:], dt_[:])
                nc.scalar.activation(dt_[:], dt_[:], AF.Sqrt)

            # ------------ Stage B: GCN layers (A@(h@W) order) --------------
            # norms fold into the data path: m' = ns (.) (h@W) on the PSUM
            # exit of the W-multiply, u' = nd (.) (A@m') before tanh --
            # A stays RAW (no per-element adjacency scaling needed).
            with tc.tile_pool(name="octo", bufs=4) as opool, \
                 tc.tile_pool(name="adj", bufs=8) as rpool, \
                 tc.tile_pool(name="gwork", bufs=10) as gpool, \
                 tc.tile_pool(name="psMm", bufs=3, space="PSUM") as ppMm, \
                 tc.tile_pool(name="psMa", bufs=3, space="PSUM") as ppMa, \
                 tc.tile_pool(name="psH", bufs=2, space="PSUM") as ppH:
                hts_rot = [0]

                def octo_prologue(o):
                    st = {}
                    st["o"] = o
                    st["ns8"] = ns_all[:, o * S:(o + 1) * S]
                    st["nd8"] = nd_all[:, o * S:(o + 1) * S]
                    xT8 = opool.tile([128, S * 128], F32, tag="xT8",
                                     name=f"xT8_{o}")
                    st["xT8"] = xT8
                    # layer-major feature blocks: col = FBASE[l]+s*dout+d
                    st["feat"] = opool.tile([128, S * DLAT], F32, tag="feat",
                                            name=f"feat_{o}")
                    nc.sync.dma_start(out=xT8[:], in_=x_imgT[o])
                    at = []
                    for half in range(2):
                        g0 = o * OCT_G + half * 4
                        at4 = rpool.tile([128, 8 * NPG], F32, tag="araw",
                                         name=f"araw_{o}_{half}")
                        nc.scalar.dma_start(out=at4[:], in_=a_img[g0 // 4])
                        at.append(at4)
                    st["at"] = at
                    return st

                def layer_m(st, li):
                    o, feat, xT8 = st["o"], st["feat"], st["xT8"]
                    dout = LAYER_DOUT[li]
                    bank_m = ppMm.tile([128, 16 * dout], F32, tag="bkm",
                                       name=f"bkm_{o}_{li}")
                    st["bank_m"] = bank_m
                    if li == 0:
                        for s in range(S):
                            nc.tensor.matmul(
                                out=bank_m[:, s * dout:(s + 1) * dout],
                                lhsT=xT8[:, s * 128:(s + 1) * 128],
                                rhs=w0_s[:, :dout],
                                start=True, stop=True)
                    else:
                        pb = FBASE[li - 1]
                        wbd = LWBD[li]
                        for grp in range(4):
                            htp = ppH.tile([128, 128], F32, tag="htp",
                                           name=f"htp_{o}_{li}_{grp}")
                            nc.tensor.transpose(
                                out=htp[:],
                                in_=feat[:, pb + grp * 128:
                                         pb + grp * 128 + 128],
                                identity=identity[:])
                            hts = gpool.tile([128, 128], F32, tag="hts",
                                             name=f"hts_{o}_{li}_{grp}")
                            nc.scalar.copy(out=hts[:], in_=htp[:])
                            nc.tensor.matmul(
                                out=bank_m[:, grp * 4 * dout:
                                           (grp + 1) * 4 * dout],
                                lhsT=hts[:],
                                rhs=wbd[:, :4 * dout],
                                start=True, stop=True)

                def layer_ns(st, li):
                    o = st["o"]
                    dout = LAYER_DOUT[li]
                    m_sb = gpool.tile([128, 16 * dout], F32, tag="msb",
                                      name=f"msb_{o}_{li}")
                    st["m_sb"] = m_sb
                    nc.vector.tensor_tensor(
                        out=m_sb[:].rearrange("p (s d) -> p s d", d=dout),
                        in0=st["bank_m"][:]
                        .rearrange("p (s d) -> p s d", d=dout),
                        in1=st["ns8"]
                        .rearrange("p (s one) -> p s one", one=1)
                        .to_broadcast([128, S, dout]),
                        op=OP.mult)

                def layer_a(st, li):
                    o = st["o"]
                    dout = LAYER_DOUT[li]
                    m_sb = st["m_sb"]

                    def a_slice(q, sc, dc):
                        t = st["at"][q // 4]
                        qq = q % 4
                        base = (2 * qq + sc) * NPG + dc * 128
                        return t[:, base:base + 128]

                    bank_a = ppMa.tile([128, 16 * dout], F32, tag="bka",
                                       name=f"bka_{o}_{li}")
                    st["bank_a"] = bank_a
                    for q in range(OCT_G):
                        for dc in range(2):
                            od = (2 * q + dc) * dout
                            for sc in range(2):
                                nc.tensor.matmul(
                                    out=bank_a[:, od:od + dout],
                                    lhsT=a_slice(q, sc, dc),
                                    rhs=m_sb[:, (2 * q + sc) * dout:
                                             (2 * q + sc + 1) * dout],
                                    start=(sc == 0), stop=(sc == 1))

                def layer_nd(st, li):
                    o = st["o"]
                    dout = LAYER_DOUT[li]
                    u_sb = gpool.tile([128, 16 * dout], F32, tag="usb",
                                      name=f"usb_{o}_{li}")
                    st["u_sb"] = u_sb
                    nc.vector.tensor_tensor(
                        out=u_sb[:].rearrange("p (s d) -> p s d", d=dout),
                        in0=st["bank_a"][:]
                        .rearrange("p (s d) -> p s d", d=dout),
                        in1=st["nd8"]
                        .rearrange("p (s one) -> p s one", one=1)
                        .to_broadcast([128, S, dout]),
                        op=OP.mult)

                def layer_tanh(st, li):
                    dout = LAYER_DOUT[li]
                    fb = FBASE[li]
                    nc.scalar.activation(
                        out=st["feat"][:, fb:fb + S * dout],
                        in_=st["u_sb"][:],
                        func=AF.Tanh)

                def octo_epilogue(st):
                    o, feat = st["o"], st["feat"]
                    if n_layers < 4:
                        nc.vector.tensor_scalar(
                            out=keys_all[:, o * S:(o + 1) * S],
                            in0=feat[:, 0:S], scalar1=0.0, scalar2=None,
                            op0=OP.add)
                        return
                    # keys = max over the 97 features
                    kb = gpool.tile([128, 3 * S], F32, tag="kb",
                                    name=f"kb_{o}")
                    for l in range(3):
                        nc.vector.tensor_reduce(
                            out=kb[:, l * S:(l + 1) * S]
                            .rearrange("p (s one) -> p s one", one=1),
                            in_=feat[:, l * 512:(l + 1) * 512]
                            .rearrange("p (s d) -> p s d", d=32),
                            axis=mybir.AxisListType.X, op=OP.max)
                    nc.vector.tensor_tensor(out=kb[:, 0:S], in0=kb[:, 0:S],
                                            in1=kb[:, S:2 * S], op=OP.max)
                    nc.vector.tensor_tensor(out=kb[:, 0:S], in0=kb[:, 0:S],
                                            in1=kb[:, 2 * S:3 * S],
                                            op=OP.max)
                    nc.vector.tensor_tensor(
                        out=keys_all[:, o * S:(o + 1) * S],
                        in0=kb[:, 0:S], in1=feat[:, 1536:1552], op=OP.max)

                    # write feat rows (columns in layer-major permuted
                    # order -- harmless, the 97-sort reorders them)
                    for c in range(2):
                        for l in range(3):
                            nc.sync.dma_start(
                                out=fdv[o, c][:, :, l * 32:(l + 1) * 32],
                                in_=feat[:, l * 512:(l + 1) * 512]
                                .rearrange("p (s d) -> p s d", d=32)
                                [:, c::2, :])
                        nc.sync.dma_start(
                            out=fdv[o, c][:, :, 96:97],
                            in_=feat[:, 1536:1552][:, c::2]
                            .rearrange("p (q one) -> p q one", one=1))

                # emit octo PAIRS with layer-interleaved instruction order:
                # per-engine streams are in-order, so alternating the two
                # independent octos' stages lets each engine fill the other
                # octo's cross-engine dependency gaps.
                GRPN = 2
                o_iter = iter(range(N_OCT))
                groups = []
                rem = N_OCT
                while rem > 0:
                    g = min(GRPN, rem)
                    groups.append([next(o_iter) for _ in range(g)])
                    rem -= g
                for grp_os in groups:
                    sts = [octo_prologue(o) for o in grp_os]
                    for li in range(n_layers):
                        for fn in (layer_m, layer_ns, layer_a, layer_nd,
                                   layer_tanh):
                            for st in sts:
                                fn(st, li)
                    for st in sts:
                        octo_epilogue(st)

            run_C = stage in ("C", "C2", "full")
            run_C2 = stage in ("C2", "full")
            run_D = stage == "full"
            if stage == "B":
                nc.sync.dma_start(out=y[:, 0:1], in_=keys_all[0:1, 0:G])

            # ------------ Stage C: top-30 selection ------------------------
            if run_C:
              with tc.tile_pool(name="selp", bufs=1) as sp, \
                 tc.tile_pool(name="selps", bufs=2, space="PSUM") as spp:
                kG = []
                srtb = []
                for h in range(2):
                    tp = spp.tile([128, 128], F32, tag="ktp")
                    nc.tensor.transpose(out=tp[:],
                                        in_=keys_all[:, h::2],
                                        identity=identity[:])
                    kg = sp.tile([128, 128], F32, tag=f"kg{h}")
                    nc.scalar.copy(out=kg[:], in_=tp[:])
                    sb = [sp.tile([128, 128], F32, tag=f"s{h}{i}",
                                  name=f"sort{h}_{i}")
                          for i in range(2)]
                    nc.vector.tensor_scalar(out=sb[0][:], in0=kg[:],
                                            scalar1=0.0, scalar2=None,
                                            op0=OP.add)
                    kG.append(kg)
                    srtb.append(sb)

                def ce_steps(n):
                    steps = []
                    klog = 1
                    while (1 << klog) <= n:
                        bs = 1 << klog
                        steps.append(("flip", bs))
                        for jj in range(klog - 2, -1, -1):
                            steps.append(("plain", 1 << jj))
                        klog += 1
                    return steps

                def bitonic_pass(sb, steps, asc, engines):
                    cur = 0
                    for (kind, d) in steps:
                        a, b_ = sb[cur][:], sb[1 - cur][:]
                        if kind == "flip":
                            bs = d
                            half = bs // 2
                            ai = a.rearrange("p (b x) -> p b x", x=bs)
                            bi = b_.rearrange("p (b x) -> p b x", x=bs)
                            lo_in = ai[:, :, 0:half]
                            hi_in = ai[:, :, bs - 1:half - 1:-1]
                            lo_out = bi[:, :, 0:half]
                            hi_out = bi[:, :, bs - 1:half - 1:-1]
                        else:
                            blk = 2 * d
                            ai = a.rearrange("p (b x) -> p b x", x=blk)
                            bi = b_.rearrange("p (b x) -> p b x", x=blk)
                            lo_in = ai[:, :, 0:d]
                            hi_in = ai[:, :, d:blk]
                            lo_out = bi[:, :, 0:d]
                            hi_out = bi[:, :, d:blk]
                        lo_op, hi_op = (OP.min, OP.max) if asc \
                            else (OP.max, OP.min)
                        engines[0].tensor_tensor(out=lo_out, in0=lo_in,
                                                 in1=hi_in, op=lo_op)
                        engines[1].tensor_tensor(out=hi_out, in0=lo_in,
                                                 in1=hi_in, op=hi_op)
                        cur = 1 - cur
                    return cur

                steps128 = ce_steps(128)
                c0 = bitonic_pass(srtb[0], steps128, True,
                                  (nc.vector, nc.vector))
                c1 = bitonic_pass(srtb[1], steps128, False,
                                  (nc.vector, nc.vector))
                # merge: hi half = top-128 (bitonic)
                hi = [sp.tile([128, 128], F32, tag=f"hi{i}",
                              name=f"hi_{i}")
                      for i in range(2)]
                nc.vector.tensor_tensor(out=hi[0][:], in0=srtb[0][c0][:],
                                        in1=srtb[1][c1][:], op=OP.max)
                # descending bitonic merge of hi
                mcur = 0
                for d in (64, 32, 16, 8, 4, 2, 1):
                    blk = 2 * d
                    a, b_ = hi[mcur][:], hi[1 - mcur][:]
                    ai = a.rearrange("p (b x) -> p b x", x=blk)
                    bi = b_.rearrange("p (b x) -> p b x", x=blk)
                    nc.vector.tensor_tensor(out=bi[:, :, 0:d],
                                            in0=ai[:, :, 0:d],
                                            in1=ai[:, :, d:blk], op=OP.max)
                    nc.vector.tensor_tensor(out=bi[:, :, d:blk],
                                            in0=ai[:, :, 0:d],
                                            in1=ai[:, :, d:blk], op=OP.min)
                    mcur = 1 - mcur
                top = hi[mcur]  # [:, 0:30] descending

                # index recovery: sel = (kG == top_r) * (iota - BIGC);
                # min over nodes -> idx - BIGC
                selw = sp.tile([128, TOPK * 256], F32, tag="selw")
                for r in range(TOPK):
                    for h in range(2):
                        eng = nc.vector
                        eng.scalar_tensor_tensor(
                            out=selw[:, r * 256 + h * 128:
                                     r * 256 + (h + 1) * 128],
                            in0=kG[h][:], scalar=top[:, r:r + 1],
                            in1=(iota0_s if h == 0 else iota1_s)[:],
                            op0=OP.is_equal, op1=OP.mult)
                idxm = sp.tile([128, TOPK], F32, tag="idxm")
                nc.vector.tensor_reduce(
                    out=idxm[:].rearrange("p (r one) -> p r one", one=1),
                    in_=selw[:].rearrange("p (r n) -> p r n", n=256),
                    axis=mybir.AxisListType.X, op=OP.min)
                nc.vector.tensor_scalar(out=idxm[:], in0=idxm[:],
                                        scalar1=BIGC, scalar2=None,
                                        op0=OP.add)
                nc.vector.tensor_scalar(out=idxm[:], in0=idxm[:],
                                        scalar1=gb_s[:], scalar2=None,
                                        op0=OP.add)
                # clamp to the valid feat_d row range (turns any upstream
                # numeric surprise into a wrong row instead of an OOB DMA)
                nc.vector.tensor_scalar_max(idxm[:], idxm[:], 0.0)
                nc.vector.tensor_scalar_min(idxm[:], idxm[:],
                                            float(G * NPG - 1))
                nc.vector.tensor_scalar(out=permI[:, :TOPK], in0=idxm[:],
                                        scalar1=0.0, scalar2=None,
                                        op0=OP.add)

            if stage == "C":
                with tc.tile_pool(name="dummy", bufs=1) as dp:
                    ysC = dp.tile([128, 1], F32, tag="ysC")
                    nc.vector.tensor_scalar(out=ysC[:], in0=permI[:, 0:1],
                                            scalar1=0.0, scalar2=None,
                                            op0=OP.add)
                    nc.sync.dma_start(out=y[:, 0:1], in_=ysC[:, 0:1])

            # ------------ Stage C2: fetch top-30 rows + 97-sort ------------
            if run_C2:
              with tc.tile_pool(name="sortp", bufs=1) as spool:
                srtf = spool.tile([G, TOPK * 128], F32, tag="srtf")
                svf = srtf[:].rearrange("p (c n) -> p c n", n=128)
                for r in range(TOPK):
                    nc.gpsimd.indirect_dma_start(
                        out=svf[:, r, 0:DLAT], out_offset=None,
                        in_=feat_d[:],
                        in_offset=IndirectOffsetOnAxis(
                            ap=permI[:G, r:r + 1], axis=0))
                # bf16 copies for the feature sort (order-only precision)
                srt = [spool.tile([G, TOPK * 128], BF16, tag=f"s{i}",
                                  name=f"srt{i}")
                       for i in range(2)]
                nc.vector.memset(srt[0][:], BIG)
                sv = [t[:].rearrange("p (c n) -> p c n", n=128) for t in srt]
                nc.vector.tensor_scalar(
                    out=sv[0][:, :, 0:DLAT], in0=svf[:, :, 0:DLAT],
                    scalar1=0.0, scalar2=None, op0=OP.add)

                # two rank-blocks: conv1 of block 0 overlaps the DVE
                # sort of block 1 (PE/ACT fill the sort tail)
                cur = 0
                for cs in (slice(0, 15), slice(15, TOPK)):
                    cur = 0
                    for (kind, d) in ce_steps(128):
                        a, b_ = sv[cur], sv[1 - cur]
                        if kind == "flip":
                            bs = d
                            half = bs // 2
                            ai = a.rearrange("p c (b x) -> p c b x", x=bs)
                            bi = b_.rearrange("p c (b x) -> p c b x", x=bs)
                            lo_in = ai[:, cs, :, 0:half]
                            hi_in = ai[:, cs, :, bs - 1:half - 1:-1]
                            lo_out = bi[:, cs, :, 0:half]
                            hi_out = bi[:, cs, :, bs - 1:half - 1:-1]
                        else:
                            blk = 2 * d
                            ai = a.rearrange("p c (b x) -> p c b x", x=blk)
                            bi = b_.rearrange("p c (b x) -> p c b x", x=blk)
                            lo_in = ai[:, cs, :, 0:d]
                            hi_in = ai[:, cs, :, d:blk]
                            lo_out = bi[:, cs, :, 0:d]
                            hi_out = bi[:, cs, :, d:blk]
                        nc.vector.tensor_tensor(out=lo_out, in0=lo_in,
                                                in1=hi_in, op=OP.min)
                        nc.vector.tensor_tensor(out=hi_out, in0=lo_in,
                                                in1=hi_in, op=OP.max)
                        cur = 1 - cur

                if stage == "C2":
                    ysD = spool.tile([128, 1], F32, tag="ysD")
                    nc.vector.tensor_reduce(
                        out=ysD[:].rearrange("p (a one) -> p a one", one=1),
                        in_=sv[cur][:, 0:1, 0:DLAT],
                        axis=mybir.AxisListType.X, op=OP.max)
                    nc.sync.dma_start(out=y[:, 0:1], in_=ysD[:, 0:1])

                # ------------ Stage D: CNN + MLP ---------------------------
                if run_D:
                  with tc.tile_pool(name="cnn", bufs=2) as cp, \
                     tc.tile_pool(name="cnnp", bufs=2, space="PSUM") as cpp:
                    z1T = spool.tile([16, TOPK * G], F32, tag="z1T")
                    sfin = sv[cur]
                    for ch in range(TOPK):
                        tp = cpp.tile([128, G], BF16, tag="ctp")
                        nc.tensor.transpose(out=tp[:, :G],
                                            in_=sfin[:, ch, :],
                                            identity=identb[:G, :G])
                        ps = cp.tile([DLAT, G], BF16, tag="ps")
                        nc.scalar.copy(out=ps[:], in_=tp[:DLAT, :G])
                        zp = cpp.tile([16, G], F32, tag="zsm")
                        nc.tensor.matmul(out=zp[:], lhsT=c1w_s[:], rhs=ps[:],
                                         start=True, stop=True)
                        nc.scalar.activation(z1T[:, ch * G:(ch + 1) * G],
                                             zp[:], AF.Relu, bias=c1b_s[:])

                    z2T = spool.tile([16, 15 * G], F32, tag="z2T")
                    z1v = z1T[:].rearrange("p (c g) -> p c g", g=G)
                    nc.vector.tensor_tensor(
                        out=z2T[:].rearrange("p (c g) -> p c g", g=G),
                        in0=z1v[:, 0:30:2, :], in1=z1v[:, 1:30:2, :],
                        op=OP.max)

                    zperm = spool.tile([G, 352], F32, tag="zperm")
                    for j in range(11):
                        z3 = cpp.tile([G, 32], F32, tag="zsm")
                        for t in range(5):
                            nc.tensor.matmul(
                                out=z3[:],
                                lhsT=z2T[:, (j + t) * G:(j + t + 1) * G],
                                rhs=w2sb_s[:, 32 * t:32 * t + 32],
                                start=(t == 0), stop=False)
                        nc.tensor.matmul(out=z3[:], lhsT=ones_row[:1, :G],
                                         rhs=b2r_s[:], start=False, stop=True)
                        nc.scalar.activation(zperm[:, 32 * j:32 * j + 32],
                                             z3[:], AF.Relu)

                    zts = []
                    for c in range(3):
                        w = min(128, 352 - 128 * c)
                        tp = cpp.tile([128, G], F32, tag="ctp")
                        nc.tensor.transpose(out=tp[:w, :G],
                                            in_=zperm[:, 128 * c:128 * c + w],
                                            identity=identity[:G, :G])
                        zt = cp.tile([128, G], F32, tag=f"zt{c}")
                        nc.scalar.copy(out=zt[:w, :], in_=tp[:w, :G])
                        zts.append((zt, w))
                    upf = cpp.tile([128, G], F32, tag="fc1")
                    for c, (zt, w) in enumerate(zts):
                        nc.tensor.matmul(
                            out=upf[:],
                            lhsT=fc1w_s[:w, 128 * c:128 * c + 128],
                            rhs=zt[:w, :], start=(c == 0), stop=(c == 2))
                    us = cp.tile([128, G], F32, tag="us")
                    nc.scalar.activation(us[:], upf[:], AF.Relu,
                                         bias=fc1b_s[:])
                    ypp = cpp.tile([1, G], F32, tag="zsm")
                    nc.tensor.matmul(out=ypp[:], lhsT=fc2w_s[:], rhs=us[:],
                                     start=True, stop=True)
                    ys = cp.tile([1, G], F32, tag="ys")
                    nc.scalar.activation(ys[:], ypp[:], AF.Identity,
                                         bias=fc2b_s[:])
                    nc.sync.dma_start(out=y[:, 0:1], in_=ys[0:1, :])

    nc.compile()
    return nc


# ---------------------------------------------------------------------------
# Host-side layout preparation + sharding
# ---------------------------------------------------------------------------

def _prep_core(c, G, x_full, ew, src, dst, deg_o, deg_i):
    g0 = c * G
    nsl = slice(g0 * NPG, (g0 + G) * NPG)
    esl = slice(g0 * EPG, (g0 + G) * EPG)

    el = np.arange(G * EPG, dtype=np.int64)
    gl = el // EPG
    src_l = np.asarray(src[esl], np.int64) - g0 * NPG - gl * NPG
    dst_l = np.asarray(dst[esl], np.int64) - g0 * NPG - gl * NPG
    assert src_l.min() >= 0 and src_l.max() < NPG
    assert dst_l.min() >= 0 and dst_l.max() < NPG

    # dense adjacency image, [g*256 + src, dst], then retile to
    # [tile=4 graphs][128 p, (k=8 chunks, 256 d)] for contiguous DMA loads
    cell = (gl * NPG + src_l) * NPG + dst_l
    a_img = np.bincount(cell, weights=ew[esl].astype(np.float64),
                        minlength=G * NPG * NPG)
    a_img = a_img.reshape(G * NPG, NPG).astype(np.float32)
    a_img = np.ascontiguousarray(
        a_img.reshape(G // 4, 8, 128, NPG).transpose(0, 2, 1, 3)
        .reshape(G // 4, 128, 8 * NPG))

    # slot layout: node(o, s, p) = (o*OCT_G + s//2)*NPG + (s%2)*128 + p
    p = np.arange(128)[:, None]
    sidx = np.arange(2 * OCT_G)[None, :]
    o = np.arange(N_OCT)[:, None, None]
    node = (o * OCT_G + sidx // 2) * NPG + (sidx % 2) * 128 + p
    xc = x_full[nsl]
    # x_imgT[o][f, s*128 + p] = x[node(o,s,p), f]
    xg = xc[node]                          # [n_oct, 128(p), S, 128(f)]
    x_imgT = np.ascontiguousarray(
        xg.transpose(0, 3, 2, 1).reshape(N_OCT, 128, S * 128))

    def deg_layout(d):
        a = d[nsl][node].astype(np.float32)
        return np.ascontiguousarray(a.transpose(1, 0, 2).reshape(128, -1))

    return dict(
        x_imgT=x_imgT.astype(np.float32), a_img=a_img,
        dego=deg_layout(deg_o), degi=deg_layout(deg_i))


def _bd4(W):
    W = np.asarray(W, np.float32)
    k, d = W.shape
    out = np.zeros((4 * k, 4 * d), np.float32)
    for i in range(4):
        out[i * k:(i + 1) * k, i * d:(i + 1) * d] = W
    return np.ascontiguousarray(out)


def _prep_weights(inp):
    f32 = lambda a: np.ascontiguousarray(np.asarray(a), np.float32)
    conv1_w = np.asarray(inp["conv1_w"], np.float32)
    conv2_w = np.asarray(inp["conv2_w"], np.float32)
    fc1_w = np.asarray(inp["fc1_w"], np.float32)

    import ml_dtypes
    c1wT = np.ascontiguousarray(conv1_w[:, 0, :].T.astype(ml_dtypes.bfloat16))
    w2sb = f32(np.transpose(conv2_w, (1, 2, 0)).reshape(16, 160))
    perm = np.empty(352, np.int64)
    for c2 in range(32):
        for j in range(11):
            perm[j * 32 + c2] = c2 * 11 + j
    fc1c = fc1_w[:, perm].T  # [352, 128] K-major
    packed = np.zeros((128, 384), np.float32)
    for c in range(3):
        w = min(128, 352 - 128 * c)
        packed[:w, 128 * c:128 * c + 128] = fc1c[128 * c:128 * c + w, :]
    jj = np.arange(128, dtype=np.float32)[None, :]
    return dict(
        ident=np.eye(128, dtype=np.float32),
        iota0=np.ascontiguousarray(np.tile(jj - BIGC, (128, 1))),
        iota1=np.ascontiguousarray(np.tile(jj + 128.0 - BIGC, (128, 1))),
        gbase=(np.arange(128, dtype=np.float32) * NPG)[:, None],
        w0=f32(inp["W0"]),
        w1bd=_bd4(inp["W1"]), w2bd=_bd4(inp["W2"]), w3bd=_bd4(inp["W3"]),
        c1wT=c1wT, c1b=f32(inp["conv1_b"]).reshape(-1, 1), w2sb=w2sb,
        b2row=f32(inp["conv2_b"]).reshape(1, -1), fc1wp=packed,
        fc1b=f32(inp["fc1_b"]).reshape(-1, 1), fc2wT=f32(inp["fc2_w"].T),
        fc2b=f32(inp["fc2_b"]).reshape(-1, 1))


def make_in_maps(inputs, G, n_cores):
    feats = np.asarray(inputs["feats"], np.int64)
    node_id = np.asarray(inputs["node_id"], np.int64)
    edge_id = np.asarray(inputs["edge_id"], np.int64)
    src = np.asarray(inputs["src"], np.int64)
    dst = np.asarray(inputs["dst"], np.int64)
    ndata = np.asarray(inputs["ndata"], np.float32)
    node_emb = np.asarray(inputs["node_emb"], np.float32)
    edata = np.asarray(inputs["edata"], np.float32)
    N = feats.shape[0]

    deg_o = np.bincount(src, minlength=N).astype(np.float32)
    deg_i = np.bincount(dst, minlength=N).astype(np.float32)
    x_full = np.concatenate(
        [ndata[node_id], node_emb[feats], node_emb[N_ATTR + node_id]],
        axis=1).astype(np.float32)
    ew = edata[edge_id][:, 0]

    shared = _prep_weights(inputs)
    in_maps = []
    for c in range(n_cores):
        m = dict(shared)
        m.update(_prep_core(c, G, x_full, ew, src, dst, deg_o, deg_i))
        in_maps.append(m)
    return in_maps


_PROG_CACHE = {}


def _get_program(G, num_devices, stage="full"):
    key = (G, num_devices, stage)
    if key not in _PROG_CACHE:
        _PROG_CACHE[key] = build_program(G, num_devices, stage)
    return _PROG_CACHE[key]


def kernel(**inputs):
    G = B // N_CORES
    nc = _get_program(G, N_CORES)
    in_maps = make_in_maps(inputs, G, N_CORES)
    res = run_bass_kernel_spmd(nc, in_maps, list(range(N_CORES)))
    out = np.concatenate([res.results[c]["y"] for c in range(N_CORES)],
                         axis=0)
    return out.astype(np.float32)


# revision 4
# speedup vs baseline: 2.1876x; 1.0665x over previous
"""DGCNN forward on 8 Trainium2 NeuronCores via Bass/Tile (v2).

Sharding: data-parallel over graphs (B/8 = 128 graphs per core).

Host-side preparation is restricted to layout/index work (slicing per-core
shards, dense adjacency image encode, embedding gathers, integer degree
counts).  All model arithmetic runs on device.

v2 changes vs baseline:
  - GCN layers in A@(h@W) order: every matmul output is <=32 wide, so the
    fp32 PE cost per layer drops ~3x (no 256-wide fp32 matmul streams).
    W-multiplies and A-multiplies for a whole octo (8 graphs) accumulate
    into shared PSUM banks; ONE tanh per octo-layer with a strided output
    into the feat tile.
  - SortPooling selection: keys are transposed into [graph, node] layout
    (2 PE transposes for all 128 graphs), sorted with a cross-graph bitonic
    network (top-128 merge + descending merge), and the top-30 node indices
    are recovered by exact f32 equality match + iota/min reduction.
  - The 97-feature bitonic sort and the adjacency norm folds are split
    across the DVE and Pool(GPSIMD) engines.
"""

import sys

if "/opt/trn_rl_repo" not in sys.path:
    sys.path.insert(0, "/opt/trn_rl_repo")

import numpy as np

import concourse.bacc as bacc
import concourse.mybir as mybir
import concourse.tile as tile
from concourse.bass import IndirectOffsetOnAxis
from concourse.bass_utils import run_bass_kernel_spmd

F32 = mybir.dt.float32
BF16 = mybir.dt.bfloat16
I32 = mybir.dt.int32
AF = mybir.ActivationFunctionType
OP = mybir.AluOpType

N_ATTR = 100000
ATTR_DIM = 64
HID = 32
B = 1024
NPG = 256
EPG = 4096
TOPK = 30
DLAT = 97
N_CORES = 8
BIG = 3.0e38
BIGC = 100000.0  # index-recovery offset (exact in f32)

OCT_G = 8      # graphs per octo
N_OCT = 16     # octos per core (G=128)
S = 16         # 128-node slots per octo


# ---------------------------------------------------------------------------
# Device program
# ---------------------------------------------------------------------------

def build_program(G, num_devices, stage="full"):
    n_layers = 4
    assert G == 128
    nc = bacc.Bacc("TRN2", target_bir_lowering=False, debug=False,
                   num_devices=num_devices)

    def din(name, shape, dt=F32):
        return nc.dram_tensor(name, shape, dt, kind="ExternalInput").ap()

    x_imgT = din("x_imgT", [N_OCT, 128, S * 128])
    a_img = din("a_img", [G // 4, 128, 8 * NPG])
    dego = din("dego", [128, N_OCT * S])
    degi = din("degi", [128, N_OCT * S])
    ident = din("ident", [128, 128])
    iota0 = din("iota0", [128, 128])   # j - BIGC
    iota1 = din("iota1", [128, 128])   # 128 + j - BIGC
    gbase = din("gbase", [128, 1])     # g * NPG
    w0 = din("w0", [2 * HID + ATTR_DIM, HID])
    w1bd = din("w1bd", [128, 4 * HID])   # block-diag(W1 x4)
    w2bd = din("w2bd", [128, 4 * HID])
    w3bd = din("w3bd", [128, 4])
    c1wT = din("c1wT", [DLAT, 16], BF16)
    c1b = din("c1b", [16, 1])
    w2sb = din("w2sb", [16, 160])
    b2row = din("b2row", [1, 32])
    fc1wp = din("fc1wp", [128, 384])
    fc1b = din("fc1b", [128, 1])
    fc2wT = din("fc2wT", [128, 1])
    fc2b = din("fc2b", [1, 1])

    y = nc.dram_tensor("y", [G, 1], F32, kind="ExternalOutput").ap()
    feat_d = nc.dram_tensor("feat_d", [G * NPG, DLAT], F32).ap()
    # feat_d row (o*OCT_G*256 + q*256 + c*128 + p) view for per-octo writes
    fdv = feat_d.rearrange("(o q c p) d -> o c p q d", q=OCT_G, c=2, p=128)

    LAYER_DOUT = (HID, HID, HID, 1)
    FBASE = (0, 512, 1024, 1536)

    with tile.TileContext(nc) as tc:
        with tc.tile_pool(name="cst", bufs=1) as cst:
            def load_const(src, shape):
                t = cst.tile(shape, src.dtype, tag=f"c{src.tensor.name}")
                nc.sync.dma_start(out=t[:], in_=src)
                return t

            identity = load_const(ident[:], [128, 128])
            identb = cst.tile([128, 128], BF16, tag="identb")
            nc.vector.tensor_scalar(out=identb[:], in0=identity[:],
                                    scalar1=0.0, scalar2=None, op0=OP.add)
            iota0_s = load_const(iota0[:], [128, 128])
            iota1_s = load_const(iota1[:], [128, 128])
            gb_s = load_const(gbase[:], [128, 1])
            ones_row = cst.tile([1, 128], F32, tag="ones_row")
            nc.vector.memset(ones_row[:], 1.0)
            permI = cst.tile([128, 32], I32, tag="permI")
            keys_all = cst.tile([128, N_OCT * S], F32, tag="keys_all")

            w0_s = load_const(w0[:], [128, HID])
            w1bd_s = load_const(w1bd[:], [128, 4 * HID])
            w2bd_s = load_const(w2bd[:], [128, 4 * HID])
            w3bd_s = load_const(w3bd[:], [128, 4])
            c1w_s = load_const(c1wT[:], [DLAT, 16])
            c1b_s = load_const(c1b[:], [16, 1])
            w2sb_s = load_const(w2sb[:], [16, 160])
            b2r_s = load_const(b2row[:], [1, 32])
            fc1w_s = load_const(fc1wp[:], [128, 384])
            fc1b_s = load_const(fc1b[:], [128, 1])
            fc2w_s = load_const(fc2wT[:], [128, 1])
            fc2b_s = load_const(fc2b[:], [1, 1])
            LWBD = (None, w1bd_s, w2bd_s, w3bd_s)

            # degree norms for ALL octos upfront
            ns_all = cst.tile([128, N_OCT * S], F32, tag="ns_all")
            nd_all = cst.tile([128, N_OCT * S], F32, tag="nd_all")
            for (dsrc, dt_) in ((dego, ns_all), (degi, nd_all)):
                nc.sync.dma_start(out=dt_[:], in_=dsrc)
                nc.vector.tensor_scalar_max(dt_[:], dt_[:], 1.0)
                nc.vector.reciprocal(dt_[:], dt_[:])
                nc.scalar.activation(dt_[:], dt_[:], AF.Sqrt)

            # ------------ Stage B: GCN layers (A@(h@W) order) --------------
            # norms fold into the data path: m' = ns (.) (h@W) on the PSUM
            # exit of the W-multiply, u' = nd (.) (A@m') before tanh --
            # A stays RAW (no per-element adjacency scaling needed).
            with tc.tile_pool(name="octo", bufs=4) as opool, \
                 tc.tile_pool(name="adj", bufs=8) as rpool, \
                 tc.tile_pool(name="gwork", bufs=10) as gpool, \
                 tc.tile_pool(name="psMm", bufs=3, space="PSUM") as ppMm, \
                 tc.tile_pool(name="psMa", bufs=3, space="PSUM") as ppMa, \
                 tc.tile_pool(name="psH", bufs=2, space="PSUM") as ppH:
                hts_rot = [0]

                def octo_prologue(o):
                    st = {}
                    st["o"] = o
                    st["ns8"] = ns_all[:, o * S:(o + 1) * S]
                    st["nd8"] = nd_all[:, o * S:(o + 1) * S]
                    xT8 = opool.tile([128, S * 128], F32, tag="xT8",
                                     name=f"xT8_{o}")
                    st["xT8"] = xT8
                    # layer-major feature blocks: col = FBASE[l]+s*dout+d
                    st["feat"] = opool.tile([128, S * DLAT], F32, tag="feat",
                                            name=f"feat_{o}")
                    nc.sync.dma_start(out=xT8[:], in_=x_imgT[o])
                    at = []
                    for half in range(2):
                        g0 = o * OCT_G + half * 4
                        at4 = rpool.tile([128, 8 * NPG], F32, tag="araw",
                                         name=f"araw_{o}_{half}")
                        nc.scalar.dma_start(out=at4[:], in_=a_img[g0 // 4])
                        at.append(at4)
                    st["at"] = at
                    return st

                def layer_m(st, li):
                    o, feat, xT8 = st["o"], st["feat"], st["xT8"]
                    dout = LAYER_DOUT[li]
                    bank_m = ppMm.tile([128, 16 * dout], F32, tag="bkm",
                                       name=f"bkm_{o}_{li}")
                    st["bank_m"] = bank_m
                    if li == 0:
                        for s in range(S):
                            nc.tensor.matmul(
                                out=bank_m[:, s * dout:(s + 1) * dout],
                                lhsT=xT8[:, s * 128:(s + 1) * 128],
                                rhs=w0_s[:, :dout],
                                start=True, stop=True)
                    else:
                        pb = FBASE[li - 1]
                        wbd = LWBD[li]
                        for grp in range(4):
                            htp = ppH.tile([128, 128], F32, tag="htp",
                                           name=f"htp_{o}_{li}_{grp}")
                            nc.tensor.transpose(
                                out=htp[:],
                                in_=feat[:, pb + grp * 128:
                                         pb + grp * 128 + 128],
                                identity=identity[:])
                            hts = gpool.tile([128, 128], F32, tag="hts",
                                             name=f"hts_{o}_{li}_{grp}")
                            nc.scalar.copy(out=hts[:], in_=htp[:])
                            nc.tensor.matmul(
                                out=bank_m[:, grp * 4 * dout:
                                           (grp + 1) * 4 * dout],
                                lhsT=hts[:],
                                rhs=wbd[:, :4 * dout],
                                start=True, stop=True)

                def layer_ns(st, li):
                    o = st["o"]
                    dout = LAYER_DOUT[li]
                    m_sb = gpool.tile([128, 16 * dout], F32, tag="msb",
                                      name=f"msb_{o}_{li}")
                    st["m_sb"] = m_sb
                    nc.vector.tensor_tensor(
                        out=m_sb[:].rearrange("p (s d) -> p s d", d=dout),
                        in0=st["bank_m"][:]
                        .rearrange("p (s d) -> p s d", d=dout),
                        in1=st["ns8"]
                        .rearrange("p (s one) -> p s one", one=1)
                        .to_broadcast([128, S, dout]),
                        op=OP.mult)

                def layer_a(st, li):
                    o = st["o"]
                    dout = LAYER_DOUT[li]
                    m_sb = st["m_sb"]

                    def a_slice(q, sc, dc):
                        t = st["at"][q // 4]
                        qq = q % 4
                        base = (2 * qq + sc) * NPG + dc * 128
                        return t[:, base:base + 128]

                    bank_a = ppMa.tile([128, 16 * dout], F32, tag="bka",
                                       name=f"bka_{o}_{li}")
                    st["bank_a"] = bank_a
                    for q in range(OCT_G):
                        for dc in range(2):
                            od = (2 * q + dc) * dout
                            for sc in range(2):
                                nc.tensor.matmul(
                                    out=bank_a[:, od:od + dout],
                                    lhsT=a_slice(q, sc, dc),
                                    rhs=m_sb[:, (2 * q + sc) * dout:
                                             (2 * q + sc + 1) * dout],
                                    start=(sc == 0), stop=(sc == 1))

                def layer_nd(st, li):
                    o = st["o"]
                    dout = LAYER_DOUT[li]
                    u_sb = gpool.tile([128, 16 * dout], F32, tag="usb",
                                      name=f"usb_{o}_{li}")
                    st["u_sb"] = u_sb
                    nc.vector.tensor_tensor(
                        out=u_sb[:].rearrange("p (s d) -> p s d", d=dout),
                        in0=st["bank_a"][:]
                        .rearrange("p (s d) -> p s d", d=dout),
                        in1=st["nd8"]
                        .rearrange("p (s one) -> p s one", one=1)
                        .to_broadcast([128, S, dout]),
                        op=OP.mult)

                def layer_tanh(st, li):
                    dout = LAYER_DOUT[li]
                    fb = FBASE[li]
                    nc.scalar.activation(
                        out=st["feat"][:, fb:fb + S * dout],
                        in_=st["u_sb"][:],
                        func=AF.Tanh)

                def octo_epilogue(st):
                    o, feat = st["o"], st["feat"]
                    if n_layers < 4:
                        nc.vector.tensor_scalar(
                            out=keys_all[:, o * S:(o + 1) * S],
                            in0=feat[:, 0:S], scalar1=0.0, scalar2=None,
                            op0=OP.add)
                        return
                    # keys = max over the 97 features
                    kb = gpool.tile([128, 3 * S], F32, tag="kb",
                                    name=f"kb_{o}")
                    for l in range(3):
                        nc.vector.tensor_reduce(
                            out=kb[:, l * S:(l + 1) * S]
                            .rearrange("p (s one) -> p s one", one=1),
                            in_=feat[:, l * 512:(l + 1) * 512]
                            .rearrange("p (s d) -> p s d", d=32),
                            axis=mybir.AxisListType.X, op=OP.max)
                    nc.vector.tensor_tensor(out=kb[:, 0:S], in0=kb[:, 0:S],
                                            in1=kb[:, S:2 * S], op=OP.max)
                    nc.vector.tensor_tensor(out=kb[:, 0:S], in0=kb[:, 0:S],
                                            in1=kb[:, 2 * S:3 * S],
                                            op=OP.max)
                    nc.vector.tensor_tensor(
                        out=keys_all[:, o * S:(o + 1) * S],
                        in0=kb[:, 0:S], in1=feat[:, 1536:1552], op=OP.max)

                    # write feat rows (columns in layer-major permuted
                    # order -- harmless, the 97-sort reorders them)
                    for c in range(2):
                        for l in range(3):
                            nc.sync.dma_start(
                                out=fdv[o, c][:, :, l * 32:(l + 1) * 32],
                                in_=feat[:, l * 512:(l + 1) * 512]
                                .rearrange("p (s d) -> p s d", d=32)
                                [:, c::2, :])
                        nc.sync.dma_start(
                            out=fdv[o, c][:, :, 96:97],
                            in_=feat[:, 1536:1552][:, c::2]
                            .rearrange("p (q one) -> p q one", one=1))

                # emit octo PAIRS with layer-interleaved instruction order:
                # per-engine streams are in-order, so alternating the two
                # independent octos' stages lets each engine fill the other
                # octo's cross-engine dependency gaps.
                GRPN = 2
                o_iter = iter(range(N_OCT))
                groups = []
                rem = N_OCT
                while rem > 0:
                    g = min(GRPN, rem)
                    groups.append([next(o_iter) for _ in range(g)])
                    rem -= g
                for grp_os in groups:
                    sts = [octo_prologue(o) for o in grp_os]
                    for li in range(n_layers):
                        for fn in (layer_m, layer_ns, layer_a, layer_nd,
                                   layer_tanh):
                            for st in sts:
                                fn(st, li)
                    for st in sts:
                        octo_epilogue(st)

            run_C = stage in ("C", "C2", "full")
            run_C2 = stage in ("C2", "full")
            run_D = stage == "full"
            if stage == "B":
                nc.sync.dma_start(out=y[:, 0:1], in_=keys_all[0:1, 0:G])

            # ------------ Stage C: top-30 selection ------------------------
            if run_C:
              with tc.tile_pool(name="selp", bufs=1) as sp, \
                 tc.tile_pool(name="selps", bufs=2, space="PSUM") as spp:
                # both node-halves in ONE [128, 256] tile; sort both
                # ascending with shared [128,2,x] ops, then a reversed-AP
                # merge extracts the (bitonic) top-128.
                kgw = sp.tile([128, 256], F32, tag="kgw")
                sbw = [sp.tile([128, 256], F32, tag=f"sbw{i}",
                               name=f"sortw_{i}")
                       for i in range(2)]
                for h in range(2):
                    tp = spp.tile([128, 128], F32, tag="ktp")
                    nc.tensor.transpose(out=tp[:],
                                        in_=keys_all[:, h::2],
                                        identity=identity[:])
                    nc.scalar.copy(out=kgw[:, h * 128:(h + 1) * 128],
                                   in_=tp[:])
                kG = [kgw[:, 0:128], kgw[:, 128:256]]
                nc.vector.tensor_scalar(out=sbw[0][:], in0=kgw[:],
                                        scalar1=0.0, scalar2=None,
                                        op0=OP.add)

                def ce_steps(n):
                    steps = []
                    klog = 1
                    while (1 << klog) <= n:
                        bs = 1 << klog
                        steps.append(("flip", bs))
                        for jj in range(klog - 2, -1, -1):
                            steps.append(("plain", 1 << jj))
                        klog += 1
                    return steps

                def bitonic_pass(sb, steps, asc, engines):
                    cur = 0
                    for (kind, d) in steps:
                        a, b_ = sb[cur][:], sb[1 - cur][:]
                        if kind == "flip":
                            bs = d
                            half = bs // 2
                            ai = a.rearrange("p (b x) -> p b x", x=bs)
                            bi = b_.rearrange("p (b x) -> p b x", x=bs)
                            lo_in = ai[:, :, 0:half]
                            hi_in = ai[:, :, bs - 1:half - 1:-1]
                            lo_out = bi[:, :, 0:half]
                            hi_out = bi[:, :, bs - 1:half - 1:-1]
                        else:
                            blk = 2 * d
                            ai = a.rearrange("p (b x) -> p b x", x=blk)
                            bi = b_.rearrange("p (b x) -> p b x", x=blk)
                            lo_in = ai[:, :, 0:d]
                            hi_in = ai[:, :, d:blk]
                            lo_out = bi[:, :, 0:d]
                            hi_out = bi[:, :, d:blk]
                        lo_op, hi_op = (OP.min, OP.max) if asc \
                            else (OP.max, OP.min)
                        engines[0].tensor_tensor(out=lo_out, in0=lo_in,
                                                 in1=hi_in, op=lo_op)
                        engines[1].tensor_tensor(out=hi_out, in0=lo_in,
                                                 in1=hi_in, op=hi_op)
                        cur = 1 - cur
                    return cur

                steps128 = ce_steps(128)
                cw = 0
                for (kind, d) in steps128:
                    a, b_ = sbw[cw][:], sbw[1 - cw][:]
                    if kind == "flip":
                        bs = d
                        half = bs // 2
                        ai = a.rearrange("p (h b x) -> p h b x", h=2, x=bs)
                        bi = b_.rearrange("p (h b x) -> p h b x", h=2, x=bs)
                        lo_in = ai[:, :, :, 0:half]
                        hi_in = ai[:, :, :, bs - 1:half - 1:-1]
                        lo_out = bi[:, :, :, 0:half]
                        hi_out = bi[:, :, :, bs - 1:half - 1:-1]
                    else:
                        blk = 2 * d
                        ai = a.rearrange("p (h b x) -> p h b x", h=2, x=blk)
                        bi = b_.rearrange("p (h b x) -> p h b x", h=2, x=blk)
                        lo_in = ai[:, :, :, 0:d]
                        hi_in = ai[:, :, :, d:blk]
                        lo_out = bi[:, :, :, 0:d]
                        hi_out = bi[:, :, :, d:blk]
                    nc.vector.tensor_tensor(out=lo_out, in0=lo_in,
                                            in1=hi_in, op=OP.min)
                    nc.vector.tensor_tensor(out=hi_out, in0=lo_in,
                                            in1=hi_in, op=OP.max)
                    cw = 1 - cw
                # merge: hi[i] = max(s0[i], s1[127-i]) -> bitonic top-128
                hi = [sp.tile([128, 128], F32, tag=f"hi{i}",
                              name=f"hi_{i}")
                      for i in range(2)]
                nc.vector.tensor_tensor(
                    out=hi[0][:], in0=sbw[cw][:, 0:128],
                    in1=sbw[cw][:, 255:127:-1], op=OP.max)
                # descending bitonic merge of hi
                mcur = 0
                for d in (64, 32, 16, 8, 4, 2, 1):
                    blk = 2 * d
                    a, b_ = hi[mcur][:], hi[1 - mcur][:]
                    ai = a.rearrange("p (b x) -> p b x", x=blk)
                    bi = b_.rearrange("p (b x) -> p b x", x=blk)
                    nc.vector.tensor_tensor(out=bi[:, :, 0:d],
                                            in0=ai[:, :, 0:d],
                                            in1=ai[:, :, d:blk], op=OP.max)
                    nc.vector.tensor_tensor(out=bi[:, :, d:blk],
                                            in0=ai[:, :, 0:d],
                                            in1=ai[:, :, d:blk], op=OP.min)
                    mcur = 1 - mcur
                top = hi[mcur]  # [:, 0:30] descending

                # index recovery: sel = (kG == top_r) * (iota - BIGC);
                # min over nodes -> idx - BIGC
                selw = sp.tile([128, TOPK * 256], F32, tag="selw")
                for r in range(TOPK):
                    for h in range(2):
                        eng = nc.vector
                        eng.scalar_tensor_tensor(
                            out=selw[:, r * 256 + h * 128:
                                     r * 256 + (h + 1) * 128],
                            in0=kG[h][:], scalar=top[:, r:r + 1],
                            in1=(iota0_s if h == 0 else iota1_s)[:],
                            op0=OP.is_equal, op1=OP.mult)
                idxm = sp.tile([128, TOPK], F32, tag="idxm")
                nc.vector.tensor_reduce(
                    out=idxm[:].rearrange("p (r one) -> p r one", one=1),
                    in_=selw[:].rearrange("p (r n) -> p r n", n=256),
                    axis=mybir.AxisListType.X, op=OP.min)
                nc.vector.tensor_scalar(out=idxm[:], in0=idxm[:],
                                        scalar1=BIGC, scalar2=None,
                                        op0=OP.add)
                nc.vector.tensor_scalar(out=idxm[:], in0=idxm[:],
                                        scalar1=gb_s[:], scalar2=None,
                                        op0=OP.add)
                # clamp to the valid feat_d row range (turns any upstream
                # numeric surprise into a wrong row instead of an OOB DMA)
                nc.vector.tensor_scalar_max(idxm[:], idxm[:], 0.0)
                nc.vector.tensor_scalar_min(idxm[:], idxm[:],
                                            float(G * NPG - 1))
                nc.vector.tensor_scalar(out=permI[:, :TOPK], in0=idxm[:],
                                        scalar1=0.0, scalar2=None,
                                        op0=OP.add)

            if stage == "C":
                with tc.tile_pool(name="dummy", bufs=1) as dp:
                    ysC = dp.tile([128, 1], F32, tag="ysC")
                    nc.vector.tensor_scalar(out=ysC[:], in0=permI[:, 0:1],
                                            scalar1=0.0, scalar2=None,
                                            op0=OP.add)
                    nc.sync.dma_start(out=y[:, 0:1], in_=ysC[:, 0:1])

            # ------------ Stage C2: fetch top-30 rows + 97-sort ------------
            if run_C2:
              with tc.tile_pool(name="sortp", bufs=1) as spool:
                srtf = spool.tile([G, TOPK * 128], F32, tag="srtf")
                svf = srtf[:].rearrange("p (c n) -> p c n", n=128)
                for r in range(TOPK):
                    nc.gpsimd.indirect_dma_start(
                        out=svf[:, r, 0:DLAT], out_offset=None,
                        in_=feat_d[:],
                        in_offset=IndirectOffsetOnAxis(
                            ap=permI[:G, r:r + 1], axis=0))
                # bf16 copies for the feature sort (order-only precision)
                srt = [spool.tile([G, TOPK * 128], BF16, tag=f"s{i}",
                                  name=f"srt{i}")
                       for i in range(2)]
                nc.vector.memset(srt[0][:], BIG)
                sv = [t[:].rearrange("p (c n) -> p c n", n=128) for t in srt]
                for cs in (slice(0, 15), slice(15, TOPK)):
                    nc.vector.tensor_scalar(
                        out=sv[0][:, cs, 0:DLAT], in0=svf[:, cs, 0:DLAT],
                        scalar1=0.0, scalar2=None, op0=OP.add)

                # two rank-blocks: conv1 of block 0 overlaps the DVE
                # sort of block 1 (PE/ACT fill the sort tail)
                cur = 0
                for cs in (slice(0, 15), slice(15, TOPK)):
                    cur = 0
                    for (kind, d) in ce_steps(128):
                        a, b_ = sv[cur], sv[1 - cur]
                        if kind == "flip":
                            bs = d
                            half = bs // 2
                            ai = a.rearrange("p c (b x) -> p c b x", x=bs)
                            bi = b_.rearrange("p c (b x) -> p c b x", x=bs)
                            lo_in = ai[:, cs, :, 0:half]
                            hi_in = ai[:, cs, :, bs - 1:half - 1:-1]
                            lo_out = bi[:, cs, :, 0:half]
                            hi_out = bi[:, cs, :, bs - 1:half - 1:-1]
                        else:
                            blk = 2 * d
                            ai = a.rearrange("p c (b x) -> p c b x", x=blk)
                            bi = b_.rearrange("p c (b x) -> p c b x", x=blk)
                            lo_in = ai[:, cs, :, 0:d]
                            hi_in = ai[:, cs, :, d:blk]
                            lo_out = bi[:, cs, :, 0:d]
                            hi_out = bi[:, cs, :, d:blk]
                        nc.vector.tensor_tensor(out=lo_out, in0=lo_in,
                                                in1=hi_in, op=OP.min)
                        nc.vector.tensor_tensor(out=hi_out, in0=lo_in,
                                                in1=hi_in, op=OP.max)
                        cur = 1 - cur

                if stage == "C2":
                    ysD = spool.tile([128, 1], F32, tag="ysD")
                    nc.vector.tensor_reduce(
                        out=ysD[:].rearrange("p (a one) -> p a one", one=1),
                        in_=sv[cur][:, 0:1, 0:DLAT],
                        axis=mybir.AxisListType.X, op=OP.max)
                    nc.sync.dma_start(out=y[:, 0:1], in_=ysD[:, 0:1])

                # ------------ Stage D: CNN + MLP ---------------------------
                if run_D:
                  with tc.tile_pool(name="cnn", bufs=2) as cp, \
                     tc.tile_pool(name="cnnp", bufs=2, space="PSUM") as cpp:
                    z1T = spool.tile([16, TOPK * G], F32, tag="z1T")
                    sfin = sv[cur]
                    for ch in range(TOPK):
                        tp = cpp.tile([128, G], BF16, tag="ctp")
                        nc.tensor.transpose(out=tp[:, :G],
                                            in_=sfin[:, ch, :],
                                            identity=identb[:G, :G])
                        ps = cp.tile([DLAT, G], BF16, tag="ps")
                        nc.scalar.copy(out=ps[:], in_=tp[:DLAT, :G])
                        zp = cpp.tile([16, G], F32, tag="zsm")
                        nc.tensor.matmul(out=zp[:], lhsT=c1w_s[:], rhs=ps[:],
                                         start=True, stop=True)
                        nc.scalar.activation(z1T[:, ch * G:(ch + 1) * G],
                                             zp[:], AF.Relu, bias=c1b_s[:])

                    z2T = spool.tile([16, 15 * G], F32, tag="z2T")
                    z1v = z1T[:].rearrange("p (c g) -> p c g", g=G)
                    nc.vector.tensor_tensor(
                        out=z2T[:].rearrange("p (c g) -> p c g", g=G),
                        in0=z1v[:, 0:30:2, :], in1=z1v[:, 1:30:2, :],
                        op=OP.max)

                    zperm = spool.tile([G, 352], F32, tag="zperm")
                    for j in range(11):
                        z3 = cpp.tile([G, 32], F32, tag="zsm")
                        for t in range(5):
                            nc.tensor.matmul(
                                out=z3[:],
                                lhsT=z2T[:, (j + t) * G:(j + t + 1) * G],
                                rhs=w2sb_s[:, 32 * t:32 * t + 32],
                                start=(t == 0), stop=False)
                        nc.tensor.matmul(out=z3[:], lhsT=ones_row[:1, :G],
                                         rhs=b2r_s[:], start=False, stop=True)
                        nc.scalar.activation(zperm[:, 32 * j:32 * j + 32],
                                             z3[:], AF.Relu)

                    zts = []
                    for c in range(3):
                        w = min(128, 352 - 128 * c)
                        tp = cpp.tile([128, G], F32, tag="ctp")
                        nc.tensor.transpose(out=tp[:w, :G],
                                            in_=zperm[:, 128 * c:128 * c + w],
                                            identity=identity[:G, :G])
                        zt = cp.tile([128, G], F32, tag=f"zt{c}")
                        nc.scalar.copy(out=zt[:w, :], in_=tp[:w, :G])
                        zts.append((zt, w))
                    upf = cpp.tile([128, G], F32, tag="fc1")
                    for c, (zt, w) in enumerate(zts):
                        nc.tensor.matmul(
                            out=upf[:],
                            lhsT=fc1w_s[:w, 128 * c:128 * c + 128],
                            rhs=zt[:w, :], start=(c == 0), stop=(c == 2))
                    us = cp.tile([128, G], F32, tag="us")
                    nc.scalar.activation(us[:], upf[:], AF.Relu,
                                         bias=fc1b_s[:])
                    ypp = cpp.tile([1, G], F32, tag="zsm")
                    nc.tensor.matmul(out=ypp[:], lhsT=fc2w_s[:], rhs=us[:],
                                     start=True, stop=True)
                    ys = cp.tile([1, G], F32, tag="ys")
                    nc.scalar.activation(ys[:], ypp[:], AF.Identity,
                                         bias=fc2b_s[:])
                    nc.sync.dma_start(out=y[:, 0:1], in_=ys[0:1, :])

    nc.compile()
    return nc


# ---------------------------------------------------------------------------
# Host-side layout preparation + sharding
# ---------------------------------------------------------------------------

def _prep_core(c, G, x_full, ew, src, dst, deg_o, deg_i):
    g0 = c * G
    nsl = slice(g0 * NPG, (g0 + G) * NPG)
    esl = slice(g0 * EPG, (g0 + G) * EPG)

    el = np.arange(G * EPG, dtype=np.int64)
    gl = el // EPG
    src_l = np.asarray(src[esl], np.int64) - g0 * NPG - gl * NPG
    dst_l = np.asarray(dst[esl], np.int64) - g0 * NPG - gl * NPG
    assert src_l.min() >= 0 and src_l.max() < NPG
    assert dst_l.min() >= 0 and dst_l.max() < NPG

    # dense adjacency image, [g*256 + src, dst], then retile to
    # [tile=4 graphs][128 p, (k=8 chunks, 256 d)] for contiguous DMA loads
    cell = (gl * NPG + src_l) * NPG + dst_l
    a_img = np.bincount(cell, weights=ew[esl].astype(np.float64),
                        minlength=G * NPG * NPG)
    a_img = a_img.reshape(G * NPG, NPG).astype(np.float32)
    a_img = np.ascontiguousarray(
        a_img.reshape(G // 4, 8, 128, NPG).transpose(0, 2, 1, 3)
        .reshape(G // 4, 128, 8 * NPG))

    # slot layout: node(o, s, p) = (o*OCT_G + s//2)*NPG + (s%2)*128 + p
    p = np.arange(128)[:, None]
    sidx = np.arange(2 * OCT_G)[None, :]
    o = np.arange(N_OCT)[:, None, None]
    node = (o * OCT_G + sidx // 2) * NPG + (sidx % 2) * 128 + p
    xc = x_full[nsl]
    # x_imgT[o][f, s*128 + p] = x[node(o,s,p), f]
    xg = xc[node]                          # [n_oct, 128(p), S, 128(f)]
    x_imgT = np.ascontiguousarray(
        xg.transpose(0, 3, 2, 1).reshape(N_OCT, 128, S * 128))

    def deg_layout(d):
        a = d[nsl][node].astype(np.float32)
        return np.ascontiguousarray(a.transpose(1, 0, 2).reshape(128, -1))

    return dict(
        x_imgT=x_imgT.astype(np.float32), a_img=a_img,
        dego=deg_layout(deg_o), degi=deg_layout(deg_i))


def _bd4(W):
    W = np.asarray(W, np.float32)
    k, d = W.shape
    out = np.zeros((4 * k, 4 * d), np.float32)
    for i in range(4):
        out[i * k:(i + 1) * k, i * d:(i + 1) * d] = W
    return np.ascontiguousarray(out)


def _prep_weights(inp):
    f32 = lambda a: np.ascontiguousarray(np.asarray(a), np.float32)
    conv1_w = np.asarray(inp["conv1_w"], np.float32)
    conv2_w = np.asarray(inp["conv2_w"], np.float32)
    fc1_w = np.asarray(inp["fc1_w"], np.float32)

    import ml_dtypes
    c1wT = np.ascontiguousarray(conv1_w[:, 0, :].T.astype(ml_dtypes.bfloat16))
    w2sb = f32(np.transpose(conv2_w, (1, 2, 0)).reshape(16, 160))
    perm = np.empty(352, np.int64)
    for c2 in range(32):
        for j in range(11):
            perm[j * 32 + c2] = c2 * 11 + j
    fc1c = fc1_w[:, perm].T  # [352, 128] K-major
    packed = np.zeros((128, 384), np.float32)
    for c in range(3):
        w = min(128, 352 - 128 * c)
        packed[:w, 128 * c:128 * c + 128] = fc1c[128 * c:128 * c + w, :]
    jj = np.arange(128, dtype=np.float32)[None, :]
    return dict(
        ident=np.eye(128, dtype=np.float32),
        iota0=np.ascontiguousarray(np.tile(jj - BIGC, (128, 1))),
        iota1=np.ascontiguousarray(np.tile(jj + 128.0 - BIGC, (128, 1))),
        gbase=(np.arange(128, dtype=np.float32) * NPG)[:, None],
        w0=f32(inp["W0"]),
        w1bd=_bd4(inp["W1"]), w2bd=_bd4(inp["W2"]), w3bd=_bd4(inp["W3"]),
        c1wT=c1wT, c1b=f32(inp["conv1_b"]).reshape(-1, 1), w2sb=w2sb,
        b2row=f32(inp["conv2_b"]).reshape(1, -1), fc1wp=packed,
        fc1b=f32(inp["fc1_b"]).reshape(-1, 1), fc2wT=f32(inp["fc2_w"].T),
        fc2b=f32(inp["fc2_b"]).reshape(-1, 1))


def make_in_maps(inputs, G, n_cores):
    feats = np.asarray(inputs["feats"], np.int64)
    node_id = np.asarray(inputs["node_id"], np.int64)
    edge_id = np.asarray(inputs["edge_id"], np.int64)
    src = np.asarray(inputs["src"], np.int64)
    dst = np.asarray(inputs["dst"], np.int64)
    ndata = np.asarray(inputs["ndata"], np.float32)
    node_emb = np.asarray(inputs["node_emb"], np.float32)
    edata = np.asarray(inputs["edata"], np.float32)
    N = feats.shape[0]

    deg_o = np.bincount(src, minlength=N).astype(np.float32)
    deg_i = np.bincount(dst, minlength=N).astype(np.float32)
    x_full = np.concatenate(
        [ndata[node_id], node_emb[feats], node_emb[N_ATTR + node_id]],
        axis=1).astype(np.float32)
    ew = edata[edge_id][:, 0]

    shared = _prep_weights(inputs)
    in_maps = []
    for c in range(n_cores):
        m = dict(shared)
        m.update(_prep_core(c, G, x_full, ew, src, dst, deg_o, deg_i))
        in_maps.append(m)
    return in_maps


_PROG_CACHE = {}


def _get_program(G, num_devices, stage="full"):
    key = (G, num_devices, stage)
    if key not in _PROG_CACHE:
        _PROG_CACHE[key] = build_program(G, num_devices, stage)
    return _PROG_CACHE[key]


def kernel(**inputs):
    G = B // N_CORES
    nc = _get_program(G, N_CORES)
    in_maps = make_in_maps(inputs, G, N_CORES)
    res = run_bass_kernel_spmd(nc, in_maps, list(range(N_CORES)))
    out = np.concatenate([res.results[c]["y"] for c in range(N_CORES)],
                         axis=0)
    return out.astype(np.float32)
